# revision 7
# baseline (speedup 1.0000x reference)
"""Trainium2 Bass kernel for the masked-autoencoder SAGE GNN problem.

Strategy (8 NeuronCores, node-sharded by dst range):
- Host (numpy, index/layout glue): sort edges by dst, bucket to 8 cores
  (12500 nodes each), lay per-edge messages out into a padded
  [128 part, tiles, 8 slots, 24 feat] stream per core (pad-to-8 per node).
- Device per core: segment-sum via (a) 8->1 slot reduce on DVE,
  (b) per-128-sub one-hot aggregation matmuls on PE accumulating whole
  128-node windows in PSUM; then norm scaling, PE transpose to
  feature-major, and the full 6-matmul MLP pipeline (centered-weight
  LayerNorm trick: mean removed exactly inside the matmul; var via
  ACT square + ones-matmul; rstd broadcast via PE ones outer product).
- Outputs per core: n_scores_raw.T [16, 12500] and recon.T [10, 12500];
  host applies the (g, beta) affine of the final LayerNorm and the
  mask_nodes row-gather.
"""

import sys

sys.path.insert(0, "/opt/trn_rl_repo")

import numpy as np

import concourse.bass as bass
import concourse.bacc as bacc
import concourse.mybir as mybir
from concourse.tile import TileContext
from concourse.bass_utils import run_bass_kernel_spmd
from concourse.masks import make_identity

F32 = mybir.dt.float32
F32R = mybir.dt.float32r

N = 100000
E = 3200000
GEO = 10
EFD = 14          # edge-feature dim: dist(1) angle(1) feat(8) disc(4)
MSGD = 24         # GEO + EFD
NCORES = 8
NPC = N // NCORES         # 12500 nodes per core
WPC = (NPC + 127) // 128  # 98 windows of 128 nodes
NPAD = WPC * 128          # 12544
LN_EPS = 1e-5
DIMS = (10, 256, 512)
NSC = 16                  # node classes
CHUNK_W = 2               # windows per phase-A chunk
NODE_CHUNK = 512          # nodes per phase-B chunk

_PROG_CACHE = {}


# --------------------------------------------------------------------------
# host-side preparation
# --------------------------------------------------------------------------

def _center(W, b):
    """Fold LayerNorm mean-subtraction into weights: rows of result produce
    z - mean_over_rows(z) exactly."""
    Wc = W - W.mean(axis=0, keepdims=True)
    bc = b - b.mean()
    return Wc.astype(np.float32), bc.astype(np.float32)


def _prepare(inputs):
    x = np.asarray(inputs["x"], np.float32)
    distance = np.asarray(inputs["distance"], np.float32)
    angle = np.asarray(inputs["angle"], np.float32)
    feat = np.asarray(inputs["feat"], np.float32)
    disc = np.asarray(inputs["disc"], np.float32)
    norm = np.asarray(inputs["norm"], np.float32)
    src = np.asarray(inputs["src"], np.int64)
    dst = np.asarray(inputs["dst"], np.int64)
    mask_nodes = np.asarray(inputs["mask_nodes"], np.int64)
    token = np.asarray(inputs["enc_mask_token"], np.float32)

    order = np.argsort(dst, kind="stable")
    dst_s = dst[order]
    bounds = np.searchsorted(dst_s, np.arange(NCORES + 1) * NPC)
    deg_all = np.bincount(dst, minlength=N).astype(np.int64)
    csub_all = (deg_all + 7) // 8             # subs per node (0 for deg 0)

    # windows: per core, per 128-node window
    csub_pad = np.zeros(NCORES * NPAD, np.int64)
    csub_pad.reshape(NCORES, NPAD)[:, :NPC] = csub_all.reshape(NCORES, NPC)
    s_w = csub_pad.reshape(NCORES, WPC, 128).sum(axis=2)     # [8, 98]
    T_W = int(((s_w + 127) // 128).max())
    n_tiles = WPC * T_W

    # masked x for encoder input
    out_x = x.copy()
    out_x[mask_nodes] = token[0]

    per_core = []
    cum_all = np.concatenate([[0], np.cumsum(deg_all)])
    for c in range(NCORES):
        lo, hi = int(bounds[c]), int(bounds[c + 1])
        e_ids = order[lo:hi]
        deg = deg_all[c * NPC:(c + 1) * NPC]
        csub = csub_all[c * NPC:(c + 1) * NPC]
        S_real = int(csub.sum())
        nodes_rep = np.repeat(np.arange(NPC), csub)          # node of each sub
        # k index of sub within its node
        sub_start = np.concatenate([[0], np.cumsum(csub)])
        sub_k = np.arange(S_real) - sub_start[nodes_rep]
        win_of_sub = nodes_rep // 128
        # rank of sub within window
        wsum = csub_pad.reshape(NCORES, WPC, 128)[c].sum(axis=1)
        win_start = np.concatenate([[0], np.cumsum(wsum)])
        pos_in_win = np.arange(S_real) - win_start[win_of_sub]
        t_in_w = pos_in_win // 128
        p_of_sub = (pos_in_win % 128).astype(np.int64)
        tile_of_sub = win_of_sub * T_W + t_in_w

        jloc = np.full((128, n_tiles), -1.0, np.float32)
        jloc[p_of_sub, tile_of_sub] = (nodes_rep % 128).astype(np.float32)

        # slot-level edge assignment
        sub_rep = np.repeat(np.arange(S_real), 8)
        j2 = np.tile(np.arange(8), S_real)
        node_rep8 = nodes_rep[sub_rep]
        e_pos = deg.cumsum()[node_rep8] - deg[node_rep8] + sub_k[sub_rep] * 8 + j2
        valid = e_pos < (deg.cumsum())[node_rep8]
        ev = e_ids[e_pos[valid]]              # global edge ids per valid slot

        msg = np.zeros((128, n_tiles, 8, MSGD), np.float32)
        pv = p_of_sub[sub_rep][valid]
        tv = tile_of_sub[sub_rep][valid]
        jv = j2[valid]
        msg[pv, tv, jv, 0:GEO] = x[src[ev]]
        msg[pv, tv, jv, GEO + 0] = distance[ev]
        msg[pv, tv, jv, GEO + 1] = angle[ev]
        msg[pv, tv, jv, GEO + 2:GEO + 10] = feat[ev]
        msg[pv, tv, jv, GEO + 10:GEO + 14] = disc[ev]

        norm_nm = np.zeros((128, WPC), np.float32)
        nc_pad = np.zeros(NPAD, np.float32)
        nc_pad[:NPC] = norm[c * NPC:(c + 1) * NPC, 0]
        norm_nm[:, :] = nc_pad.reshape(WPC, 128).T

        xhatT = out_x[c * NPC:(c + 1) * NPC].T.copy()

        per_core.append(dict(
            msg=msg.reshape(128, n_tiles * 8 * MSGD),
            jloc=jloc,
            norm_nm=norm_nm,
            xhatT=xhatT,
        ))

    # ---- weights (shared across cores) ----
    w = {}
    e0W, e0b = _center(inputs["enc0_W"], inputs["enc0_b"])
    e1W, e1b = _center(inputs["enc1_W"], inputs["enc1_b"])
    d0W, d0b = _center(inputs["dec0_W"], inputs["dec0_b"])
    d1W, d1b = _center(inputs["dec1_W"], inputs["dec1_b"])
    npW, npb = _center(inputs["np_W"], inputs["np_b"])
    e2dW = np.asarray(inputs["e2d_W"], np.float32)

    # per-layer: h-part K-tiles + ah-tile rows [bias ; Wa^T] matching
    # rhs ah25 = [ones ; ah*norm] (ones row FIRST, partition 0)
    w["wt0x"] = e0W[:, 0:GEO].T.copy()                                   # [10, 256]
    w["a0"] = np.concatenate([e0b[None, :], e0W[:, GEO:34].T], axis=0)   # [25, 256]
    w["wt1"] = e1W[:, 0:256].T.copy()                                    # [256, 512]
    w["a1"] = np.concatenate([e1b[None, :], e1W[:, 256:280].T], axis=0)  # [25, 512]
    w["wtnp"] = npW.T.copy()                                             # [512, 16]
    w["anp"] = np.concatenate([npb[None, :],
                               np.zeros((MSGD, NSC), np.float32)], axis=0)
    w["wte"] = e2dW.T.copy()                                             # [512, 512]
    w["wtd0"] = d0W[:, 0:512].T.copy()                                   # [512, 256]
    w["ad0"] = np.concatenate([d0b[None, :], d0W[:, 512:536].T], axis=0)
    w["wtd1"] = d1W[:, 0:256].T.copy()                                   # [256, 10]
    w["ad1"] = np.concatenate([d1b[None, :], d1W[:, 256:280].T], axis=0)
    iota = np.tile(np.arange(128, dtype=np.float32), (128, 1))
    w["iota"] = iota

    meta = dict(T_W=T_W, n_tiles=n_tiles)
    return per_core, w, meta


# --------------------------------------------------------------------------
# device program
# --------------------------------------------------------------------------

def _build_program(T_W):
    n_tiles = WPC * T_W
    CW = CHUNK_W
    n_chunks_a = WPC // CW                      # 49
    tiles_pc = CW * T_W                         # tiles per chunk
    msg_w = n_tiles * 8 * MSGD                  # msg cols per partition

    nc = bacc.Bacc("TRN2", target_bir_lowering=False)

    msg_d = nc.declare_dram_parameter("msg", [128, msg_w], F32, isOutput=False)
    jloc_d = nc.declare_dram_parameter("jloc", [128, n_tiles], F32, isOutput=False)
    iota_d = nc.declare_dram_parameter("iota", [128, 128], F32, isOutput=False)
    norm_d = nc.declare_dram_parameter("norm_nm", [128, WPC], F32, isOutput=False)
    xhat_d = nc.declare_dram_parameter("xhatT", [GEO, NPC], F32R, isOutput=False)
    wt0x_d = nc.declare_dram_parameter("wt0x", [GEO, 256], F32R, isOutput=False)
    a0_d = nc.declare_dram_parameter("a0", [25, 256], F32R, isOutput=False)
    wt1_d = nc.declare_dram_parameter("wt1", [256, 512], F32R, isOutput=False)
    a1_d = nc.declare_dram_parameter("a1", [25, 512], F32R, isOutput=False)
    wtnp_d = nc.declare_dram_parameter("wtnp", [512, NSC], F32R, isOutput=False)
    anp_d = nc.declare_dram_parameter("anp", [25, NSC], F32R, isOutput=False)
    wte_d = nc.declare_dram_parameter("wte", [512, 512], F32R, isOutput=False)
    wtd0_d = nc.declare_dram_parameter("wtd0", [512, 256], F32R, isOutput=False)
    ad0_d = nc.declare_dram_parameter("ad0", [25, 256], F32R, isOutput=False)
    wtd1_d = nc.declare_dram_parameter("wtd1", [256, GEO], F32R, isOutput=False)
    ad1_d = nc.declare_dram_parameter("ad1", [25, GEO], F32R, isOutput=False)
    scores_d = nc.declare_dram_parameter("scoresT", [NSC, NPC], F32, isOutput=True)
    ahnT_d = nc.dram_tensor("ahnT_buf", [MSGD + 1, NPC], F32R)
    recon_d = nc.declare_dram_parameter("reconT", [GEO, NPC], F32, isOutput=True)

    with nc.allow_low_precision(reason="fp32r matmul inputs"), \
         TileContext(nc) as tc:
        with (
            tc.tile_pool(name="const", bufs=1) as cpool,
            tc.tile_pool(name="wts", bufs=1) as wpool,
            tc.tile_pool(name="persist", bufs=1) as ppool,
        ):
            # ---- constants ----
            iota_t = cpool.tile([128, 128], F32)
            nc.sync.dma_start(out=iota_t[:], in_=iota_d[:])
            ident_t = cpool.tile([128, 128], F32)
            make_identity(nc, ident_t[:])
            jloc_t = cpool.tile([128, n_tiles], F32)
            nc.sync.dma_start(out=jloc_t[:], in_=jloc_d[:])
            norm_t = cpool.tile([128, WPC], F32)
            nc.sync.dma_start(out=norm_t[:], in_=norm_d[:])
            ones_f = cpool.tile([128, 1], F32)
            nc.vector.memset(ones_f[:], 1.0)
            ones_r = cpool.tile([128, 1], F32R)
            nc.vector.tensor_copy(out=ones_r[:], in_=ones_f[:])
            eps_t = cpool.tile([1, 1], F32)
            nc.vector.memset(eps_t[:], LN_EPS)
            onesrow_f = cpool.tile([1, 128], F32)
            nc.vector.memset(onesrow_f[:], 1.0)
            onesrow_r = cpool.tile([1, 128], F32R)
            nc.vector.tensor_copy(out=onesrow_r[:], in_=onesrow_f[:])


            # ---- weights to SBUF ----
            def load_w(dram, rows, cols):
                tiles = []
                r0 = 0
                while r0 < rows:
                    r = min(128, rows - r0)
                    t = wpool.tile([r, cols], F32R, tag=f"w{dram.name}{r0}")
                    nc.sync.dma_start(out=t[:], in_=dram[r0:r0 + r, :])
                    tiles.append((t, r))
                    r0 += r
                return tiles

            wt0x = load_w(wt0x_d, GEO, 256)
            a0 = load_w(a0_d, 25, 256)
            wt1 = load_w(wt1_d, 256, 512)
            a1 = load_w(a1_d, 25, 512)
            wtnp = load_w(wtnp_d, 512, NSC)
            anp = load_w(anp_d, 25, NSC)
            wte = load_w(wte_d, 512, 512)
            wtd0 = load_w(wtd0_d, 512, 256)
            ad0 = load_w(ad0_d, 25, 256)
            wtd1 = load_w(wtd1_d, 256, GEO)
            ad1 = load_w(ad1_d, 25, GEO)

            # ================= phase A =================
            with (
                tc.tile_pool(name="msgp", bufs=3) as mpool,
                tc.tile_pool(name="subs", bufs=3) as spool,
                tc.tile_pool(name="aggp", bufs=4) as apool,
                tc.tile_pool(name="psA", bufs=4, space="PSUM") as psA,
                tc.tile_pool(name="ahp", bufs=1) as ahpool,
            ):
                ah_t = ahpool.tile([128, WPC * MSGD], F32)
                ahn_t = ahpool.tile([128, WPC * (MSGD + 1)], F32)
                for ck in range(n_chunks_a):
                    msg_t = mpool.tile([128, tiles_pc * 8 * MSGD], F32)
                    nc.sync.dma_start(
                        out=msg_t[:],
                        in_=msg_d[:, ck * tiles_pc * 8 * MSGD:
                                  (ck + 1) * tiles_pc * 8 * MSGD])
                    subs_t = spool.tile([128, tiles_pc * MSGD], F32R)
                    msg_v = msg_t[:].rearrange("p (t s f) -> p t f s",
                                               t=tiles_pc, s=8, f=MSGD)
                    nc.vector.reduce_sum(
                        out=subs_t[:].rearrange("p (t f) -> p t f",
                                                t=tiles_pc, f=MSGD),
                        in_=msg_v, axis=mybir.AxisListType.X)
                    win_p = psA.tile([128, CW * MSGD], F32, space="PSUM",
                                     tag="win")
                    for tt in range(tiles_pc):
                        w_in_c = tt // T_W
                        k_in_w = tt % T_W
                        agg_t = apool.tile([128, 128], F32R, tag="agg")
                        gtile = ck * tiles_pc + tt
                        nc.vector.tensor_scalar(
                            out=agg_t[:], in0=iota_t[:],
                            scalar1=jloc_t[:, gtile:gtile + 1], scalar2=None,
                            op0=mybir.AluOpType.is_equal)
                        nc.tensor.matmul(
                            out=win_p[:, w_in_c * MSGD:(w_in_c + 1) * MSGD],
                            lhsT=agg_t[:],
                            rhs=subs_t[:, tt * MSGD:(tt + 1) * MSGD],
                            start=(k_in_w == 0), stop=(k_in_w == T_W - 1))
                    nc.vector.tensor_copy(
                        out=ah_t[:, ck * CW * MSGD:(ck + 1) * CW * MSGD],
                        in_=win_p[:])

                # ---- ah * norm, ones column -> ahn [128, WPC*25] ----
                ahn_v = ahn_t[:].rearrange("p (w f) -> p w f", w=WPC,
                                           f=MSGD + 1)
                nc.vector.memset(ahn_v[:, :, 0:1], 1.0)
                normB_t = mpool.tile([128, WPC * MSGD], F32, tag="msg_t")
                nc.vector.tensor_copy(
                    out=normB_t[:].rearrange("p (w f) -> p w f", w=WPC, f=MSGD),
                    in_=norm_t[:].to_broadcast([128, WPC, MSGD]))
                nc.vector.tensor_tensor(
                    out=ahn_v[:, :, 1:MSGD + 1],
                    in0=ah_t[:].rearrange("p (w f) -> p w f", w=WPC, f=MSGD),
                    in1=normB_t[:].rearrange("p (w f) -> p w f", w=WPC, f=MSGD),
                    op=mybir.AluOpType.mult)

                # ---- transpose ahn -> ahnT [25, NPC] ----
                for g in range(0, WPC, 4):
                    gw = min(4, WPC - g)
                    tr_p = psA.tile([MSGD + 1, 512], F32, space="PSUM",
                                    tag="trp")
                    for k in range(gw):
                        nc.tensor.transpose(
                            out=tr_p[:, k * 128:(k + 1) * 128],
                            in_=ahn_v[:, g + k, :], identity=ident_t[:])
                    ncols = min(NPC - g * 128, gw * 128)
                    stg = spool.tile([MSGD + 1, 512], F32R, tag="stg")
                    nc.vector.tensor_copy(out=stg[:, 0:ncols],
                                          in_=tr_p[:, 0:ncols])
                    nc.sync.dma_start(
                        out=ahnT_d[:, g * 128:g * 128 + ncols],
                        in_=stg[:, 0:ncols])

            # ================= phase B =================
            with (
                tc.tile_pool(name="actp", bufs=2) as hpool,
                tc.tile_pool(name="psB", bufs=5, space="PSUM") as psB,
                tc.tile_pool(name="psS", bufs=1, space="PSUM") as psS,
            ):
                def layer(rhs_tiles, wts, fout, cn, ln_relu, out_dtype=F32R,
                          tag=""):
                    n_mt = (fout + 127) // 128
                    zs = []
                    for mt in range(n_mt):
                        m = min(128, fout - mt * 128)
                        z_p = psB.tile([128, NODE_CHUNK], F32, space="PSUM",
                                       tag="z")
                        for kt, ((rhs_ap, k), wt_ap) in enumerate(
                                zip(rhs_tiles, wts)):
                            nc.tensor.matmul(
                                out=z_p[:m, 0:cn],
                                lhsT=wt_ap[:, mt * 128:mt * 128 + m],
                                rhs=rhs_ap,
                                start=(kt == 0),
                                stop=(kt == len(rhs_tiles) - 1))
                        zs.append((z_p, m))
                    if not ln_relu:
                        outs = []
                        for mt, (z_p, m) in enumerate(zs):
                            o = hpool.tile([128, NODE_CHUNK], out_dtype,
                                           tag=f"o{tag}{mt}")
                            nc.scalar.copy(out=o[:m, 0:cn], in_=z_p[:m, 0:cn])
                            outs.append((o, m))
                        return outs
                    var_p = psS.tile([1, NODE_CHUNK], F32, space="PSUM",
                                     tag="v")
                    zsqs = []
                    for mt, (z_p, m) in enumerate(zs):
                        zsq = hpool.tile([128, NODE_CHUNK], F32R, tag="q")
                        nc.scalar.square(out=zsq[:m, 0:cn], in_=z_p[:m, 0:cn])
                        zsqs.append((zsq, m))
                    for mt, (zsq, m) in enumerate(zsqs):
                        nc.tensor.matmul(
                            out=var_p[:, 0:cn], lhsT=ones_r[:m, :],
                            rhs=zsq[:m, 0:cn],
                            start=(mt == 0), stop=(mt == len(zsqs) - 1))
                    s_t = hpool.tile([1, NODE_CHUNK], F32R, tag="s")
                    nc.scalar.activation(
                        out=s_t[:, 0:cn], in_=var_p[:, 0:cn],
                        func=mybir.ActivationFunctionType.Sqrt,
                        bias=eps_t[:1, :1], scale=1.0 / fout)
                    sB_p = psS.tile([128, NODE_CHUNK], F32, space="PSUM",
                                    tag="sb")
                    nc.tensor.matmul(out=sB_p[:, 0:cn], lhsT=onesrow_r[:, :],
                                     rhs=s_t[:, 0:cn], start=True, stop=True)
                    rstdB = hpool.tile([128, NODE_CHUNK], F32, tag="r")
                    nc.vector.reciprocal(out=rstdB[:, 0:cn], in_=sB_p[:, 0:cn])
                    outs = []
                    for mt, (z_p, m) in enumerate(zs):
                        t_t = hpool.tile([128, NODE_CHUNK], F32, tag="t")
                        nc.vector.tensor_tensor(
                            out=t_t[:m, 0:cn], in0=z_p[:m, 0:cn],
                            in1=rstdB[:m, 0:cn],
                            op=mybir.AluOpType.mult)
                        o = hpool.tile([128, NODE_CHUNK], out_dtype,
                                       tag=f"o{tag}{mt}")
                        if ln_relu == "relu":
                            nc.scalar.activation(
                                out=o[:m, 0:cn], in_=t_t[:m, 0:cn],
                                func=mybir.ActivationFunctionType.Relu)
                        else:
                            nc.scalar.copy(out=o[:m, 0:cn], in_=t_t[:m, 0:cn])
                        outs.append((o, m))
                    return outs

                n_chunks_b = (NPC + NODE_CHUNK - 1) // NODE_CHUNK
                for nck in range(n_chunks_b):
                    c0 = nck * NODE_CHUNK
                    cn = min(NODE_CHUNK, NPC - c0)
                    sl = slice(c0, c0 + cn)
                    ahn_c = hpool.tile([MSGD + 1, NODE_CHUNK], F32R, tag="ahc")
                    nc.sync.dma_start(out=ahn_c[:, 0:cn], in_=ahnT_d[:, sl])
                    xhat_c = hpool.tile([GEO, NODE_CHUNK], F32R, tag="xhc")
                    nc.sync.dma_start(out=xhat_c[:, 0:cn], in_=xhat_d[:, sl])
                    ah25 = ahn_c[0:MSGD + 1, 0:cn]

                    h1 = layer([(xhat_c[:, 0:cn], GEO), (ah25, MSGD + 1)],
                               [wt0x[0][0], a0[0][0]],
                               256, cn, "relu", tag="e0")
                    h2 = layer([(h1[0][0][:, 0:cn], 128),
                                (h1[1][0][:, 0:cn], 128),
                                (ah25, MSGD + 1)],
                               [wt1[0][0], wt1[1][0], a1[0][0]],
                               512, cn, "relu", tag="e1")
                    h2r = [(t[:, 0:cn], 128) for t, m in h2]
                    sc = layer(h2r + [(ah25, MSGD + 1)],
                               [wtnp[0][0], wtnp[1][0], wtnp[2][0],
                                wtnp[3][0], anp[0][0]],
                               NSC, cn, "ln", out_dtype=F32, tag="np")
                    nc.sync.dma_start(out=scores_d[:, sl],
                                      in_=sc[0][0][0:NSC, 0:cn])
                    rep = layer(h2r,
                                [wte[0][0], wte[1][0], wte[2][0], wte[3][0]],
                                512, cn, False, tag="ed")
                    repr_ = [(t[:, 0:cn], 128) for t, m in rep]
                    d1 = layer(repr_ + [(ah25, MSGD + 1)],
                               [wtd0[0][0], wtd0[1][0], wtd0[2][0],
                                wtd0[3][0], ad0[0][0]],
                               256, cn, "relu", tag="d0")
                    rec = layer([(d1[0][0][:, 0:cn], 128),
                                 (d1[1][0][:, 0:cn], 128),
                                 (ah25, MSGD + 1)],
                                [wtd1[0][0], wtd1[1][0], ad1[0][0]],
                                GEO, cn, "relu", out_dtype=F32, tag="d1")
                    nc.sync.dma_start(out=recon_d[:, sl],
                                      in_=rec[0][0][0:GEO, 0:cn])

    nc.compile()
    return nc


# --------------------------------------------------------------------------
# entry point
# --------------------------------------------------------------------------

def kernel(**inputs):
    import os
    per_core, w, meta = _prepare(inputs)
    T_W = meta["T_W"]
    if T_W not in _PROG_CACHE:
        _PROG_CACHE[T_W] = _build_program(T_W)
    nc = _PROG_CACHE[T_W]

    in_maps = []
    for c in range(NCORES):
        m = dict(per_core[c])
        m.update(w)
        in_maps.append(m)

    import time as _time
    trace = bool(int(os.environ.get("BASS_KERNEL_TRACE", "0")))
    t0 = _time.time()
    try:
        res = run_bass_kernel_spmd(nc, in_maps, list(range(NCORES)), trace=trace)
    except ModuleNotFoundError:
        res = run_bass_kernel_spmd(nc, in_maps, list(range(NCORES)), trace=False)
    exec_wall = _time.time() - t0
    if getattr(res, "exec_time_ns", None) is not None:
        print(f"HW exec time: {res.exec_time_ns} ns")
    else:
        print(f"HW exec time: {int(exec_wall * 1e9)} ns (wall-clock of run, no profile)")

    scores = np.concatenate([res.results[c]["scoresT"].T for c in range(NCORES)],
                            axis=0)
    recon = np.concatenate([res.results[c]["reconT"].T for c in range(NCORES)],
                           axis=0)
    g = np.asarray(inputs["np_g"], np.float32)
    beta = np.asarray(inputs["np_beta"], np.float32)
    n_scores = scores * g[None, :] + beta[None, :]
    mask_nodes = np.asarray(inputs["mask_nodes"], np.int64)
    x = np.asarray(inputs["x"], np.float32)
    x_pred = recon[mask_nodes].astype(np.float32)
    x_true = x[mask_nodes].astype(np.float32)
    return (x_pred, x_true, n_scores.astype(np.float32))


# revision 8
# speedup vs baseline: 1.8351x; 1.8351x over previous
"""Trainium2 Bass kernel for the masked-autoencoder SAGE GNN problem.

Strategy (8 NeuronCores, node-sharded by dst range):
- Host (numpy, index/layout glue): sort edges by dst, bucket to 8 cores
  (12500 nodes each), lay per-edge messages out into a padded
  [128 part, tiles, 8 slots, 24 feat] stream per core (pad-to-8 per node).
- Device per core: segment-sum via (a) 8->1 slot reduce on DVE,
  (b) per-128-sub one-hot aggregation matmuls on PE accumulating whole
  128-node windows in PSUM; then norm scaling, PE transpose to
  feature-major, and the full 6-matmul MLP pipeline (centered-weight
  LayerNorm trick: mean removed exactly inside the matmul; var via
  ACT square + ones-matmul; rstd broadcast via PE ones outer product).
- Outputs per core: n_scores_raw.T [16, 12500] and recon.T [10, 12500];
  host applies the (g, beta) affine of the final LayerNorm and the
  mask_nodes row-gather.
"""

import sys

sys.path.insert(0, "/opt/trn_rl_repo")

import numpy as np

import concourse.bass as bass
import concourse.bacc as bacc
import concourse.mybir as mybir
from concourse.tile import TileContext
from concourse.bass_utils import run_bass_kernel_spmd
from concourse.masks import make_identity

F32 = mybir.dt.float32
F32R = mybir.dt.float32r
BF16 = mybir.dt.bfloat16

N = 100000
E = 3200000
GEO = 10
EFD = 14          # edge-feature dim: dist(1) angle(1) feat(8) disc(4)
MSGD = 24         # GEO + EFD
NCORES = 8
NPC = N // NCORES         # 12500 nodes per core
WPC = (NPC + 127) // 128  # 98 windows of 128 nodes
NPAD = WPC * 128          # 12544
LN_EPS = 1e-5
DIMS = (10, 256, 512)
NSC = 16                  # node classes
CHUNK_W = 2               # windows per phase-A chunk
NODE_CHUNK = 512          # nodes per phase-B chunk

_PROG_CACHE = {}


# --------------------------------------------------------------------------
# host-side preparation
# --------------------------------------------------------------------------

def _center(W, b):
    """Fold LayerNorm mean-subtraction into weights: rows of result produce
    z - mean_over_rows(z) exactly."""
    Wc = W - W.mean(axis=0, keepdims=True)
    bc = b - b.mean()
    return Wc.astype(np.float32), bc.astype(np.float32)


def _prepare(inputs):
    x = np.asarray(inputs["x"], np.float32)
    distance = np.asarray(inputs["distance"], np.float32)
    angle = np.asarray(inputs["angle"], np.float32)
    feat = np.asarray(inputs["feat"], np.float32)
    disc = np.asarray(inputs["disc"], np.float32)
    norm = np.asarray(inputs["norm"], np.float32)
    src = np.asarray(inputs["src"], np.int64)
    dst = np.asarray(inputs["dst"], np.int64)
    mask_nodes = np.asarray(inputs["mask_nodes"], np.int64)
    token = np.asarray(inputs["enc_mask_token"], np.float32)

    order = np.argsort(dst, kind="stable")
    dst_s = dst[order]
    bounds = np.searchsorted(dst_s, np.arange(NCORES + 1) * NPC)
    deg_all = np.bincount(dst, minlength=N).astype(np.int64)
    csub_all = (deg_all + 7) // 8             # subs per node (0 for deg 0)

    # windows: per core, per 128-node window
    csub_pad = np.zeros(NCORES * NPAD, np.int64)
    csub_pad.reshape(NCORES, NPAD)[:, :NPC] = csub_all.reshape(NCORES, NPC)
    s_w = csub_pad.reshape(NCORES, WPC, 128).sum(axis=2)     # [8, 98]
    T_W = int(((s_w + 127) // 128).max())
    n_tiles = WPC * T_W

    # masked x for encoder input
    out_x = x.copy()
    out_x[mask_nodes] = token[0]

    per_core = []
    cum_all = np.concatenate([[0], np.cumsum(deg_all)])
    for c in range(NCORES):
        lo, hi = int(bounds[c]), int(bounds[c + 1])
        e_ids = order[lo:hi]
        deg = deg_all[c * NPC:(c + 1) * NPC]
        csub = csub_all[c * NPC:(c + 1) * NPC]
        S_real = int(csub.sum())
        nodes_rep = np.repeat(np.arange(NPC), csub)          # node of each sub
        # k index of sub within its node
        sub_start = np.concatenate([[0], np.cumsum(csub)])
        sub_k = np.arange(S_real) - sub_start[nodes_rep]
        win_of_sub = nodes_rep // 128
        # rank of sub within window
        wsum = csub_pad.reshape(NCORES, WPC, 128)[c].sum(axis=1)
        win_start = np.concatenate([[0], np.cumsum(wsum)])
        pos_in_win = np.arange(S_real) - win_start[win_of_sub]
        t_in_w = pos_in_win // 128
        p_of_sub = (pos_in_win % 128).astype(np.int64)
        tile_of_sub = win_of_sub * T_W + t_in_w

        jloc = np.full((128, n_tiles), -1.0, np.float32)
        jloc[p_of_sub, tile_of_sub] = (nodes_rep % 128).astype(np.float32)

        # slot-level edge assignment
        sub_rep = np.repeat(np.arange(S_real), 8)
        j2 = np.tile(np.arange(8), S_real)
        node_rep8 = nodes_rep[sub_rep]
        e_pos = deg.cumsum()[node_rep8] - deg[node_rep8] + sub_k[sub_rep] * 8 + j2
        valid = e_pos < (deg.cumsum())[node_rep8]
        ev = e_ids[e_pos[valid]]              # global edge ids per valid slot

        msg = np.zeros((128, n_tiles, 8, MSGD), np.float32)
        pv = p_of_sub[sub_rep][valid]
        tv = tile_of_sub[sub_rep][valid]
        jv = j2[valid]
        msg[pv, tv, jv, 0:GEO] = x[src[ev]]
        msg[pv, tv, jv, GEO + 0] = distance[ev]
        msg[pv, tv, jv, GEO + 1] = angle[ev]
        msg[pv, tv, jv, GEO + 2:GEO + 10] = feat[ev]
        msg[pv, tv, jv, GEO + 10:GEO + 14] = disc[ev]

        norm_nm = np.zeros((128, WPC), np.float32)
        nc_pad = np.zeros(NPAD, np.float32)
        nc_pad[:NPC] = norm[c * NPC:(c + 1) * NPC, 0]
        norm_nm[:, :] = nc_pad.reshape(WPC, 128).T

        xhatT = out_x[c * NPC:(c + 1) * NPC].T.copy()

        import ml_dtypes
        per_core.append(dict(
            msg=msg.reshape(128, n_tiles * 8 * MSGD).astype(ml_dtypes.bfloat16),
            jloc=jloc,
            norm_nm=norm_nm,
            xhatT=xhatT,
        ))

    # ---- weights (shared across cores) ----
    w = {}
    e0W, e0b = _center(inputs["enc0_W"], inputs["enc0_b"])
    e1W, e1b = _center(inputs["enc1_W"], inputs["enc1_b"])
    d0W, d0b = _center(inputs["dec0_W"], inputs["dec0_b"])
    d1W, d1b = _center(inputs["dec1_W"], inputs["dec1_b"])
    npW, npb = _center(inputs["np_W"], inputs["np_b"])
    e2dW = np.asarray(inputs["e2d_W"], np.float32)

    # per-layer: h-part K-tiles + ah-tile rows [bias ; Wa^T] matching
    # rhs ah25 = [ones ; ah*norm] (ones row FIRST, partition 0)
    w["wt0x"] = e0W[:, 0:GEO].T.copy()                                   # [10, 256]
    w["a0"] = np.concatenate([e0b[None, :], e0W[:, GEO:34].T], axis=0)   # [25, 256]
    w["wt1"] = e1W[:, 0:256].T.copy()                                    # [256, 512]
    w["a1"] = np.concatenate([e1b[None, :], e1W[:, 256:280].T], axis=0)  # [25, 512]
    w["wtnp"] = npW.T.copy()                                             # [512, 16]
    w["anp"] = np.concatenate([npb[None, :],
                               np.zeros((MSGD, NSC), np.float32)], axis=0)
    w["wte"] = e2dW.T.copy()                                             # [512, 512]
    w["wtd0"] = d0W[:, 0:512].T.copy()                                   # [512, 256]
    w["ad0"] = np.concatenate([d0b[None, :], d0W[:, 512:536].T], axis=0)
    w["wtd1"] = d1W[:, 0:256].T.copy()                                   # [256, 10]
    w["ad1"] = np.concatenate([d1b[None, :], d1W[:, 256:280].T], axis=0)
    iota = np.tile(np.arange(128, dtype=np.float32), (128, 1))
    w["iota"] = iota

    meta = dict(T_W=T_W, n_tiles=n_tiles)
    return per_core, w, meta


# --------------------------------------------------------------------------
# device program
# --------------------------------------------------------------------------

def _build_program(T_W):
    n_tiles = WPC * T_W
    CW = CHUNK_W
    n_chunks_a = WPC // CW                      # 49
    tiles_pc = CW * T_W                         # tiles per chunk
    msg_w = n_tiles * 8 * MSGD                  # msg cols per partition

    nc = bacc.Bacc("TRN2", target_bir_lowering=False)

    msg_d = nc.declare_dram_parameter("msg", [128, msg_w], BF16, isOutput=False)
    jloc_d = nc.declare_dram_parameter("jloc", [128, n_tiles], F32, isOutput=False)
    iota_d = nc.declare_dram_parameter("iota", [128, 128], F32, isOutput=False)
    norm_d = nc.declare_dram_parameter("norm_nm", [128, WPC], F32, isOutput=False)
    xhat_d = nc.declare_dram_parameter("xhatT", [GEO, NPC], F32R, isOutput=False)
    wt0x_d = nc.declare_dram_parameter("wt0x", [GEO, 256], F32R, isOutput=False)
    a0_d = nc.declare_dram_parameter("a0", [25, 256], F32R, isOutput=False)
    wt1_d = nc.declare_dram_parameter("wt1", [256, 512], F32R, isOutput=False)
    a1_d = nc.declare_dram_parameter("a1", [25, 512], F32R, isOutput=False)
    wtnp_d = nc.declare_dram_parameter("wtnp", [512, NSC], F32R, isOutput=False)
    anp_d = nc.declare_dram_parameter("anp", [25, NSC], F32R, isOutput=False)
    wte_d = nc.declare_dram_parameter("wte", [512, 512], F32R, isOutput=False)
    wtd0_d = nc.declare_dram_parameter("wtd0", [512, 256], F32R, isOutput=False)
    ad0_d = nc.declare_dram_parameter("ad0", [25, 256], F32R, isOutput=False)
    wtd1_d = nc.declare_dram_parameter("wtd1", [256, GEO], F32R, isOutput=False)
    ad1_d = nc.declare_dram_parameter("ad1", [25, GEO], F32R, isOutput=False)
    scores_d = nc.declare_dram_parameter("scoresT", [NSC, NPC], F32, isOutput=True)
    ahnT_d = nc.dram_tensor("ahnT_buf", [MSGD + 1, NPC], F32R)
    recon_d = nc.declare_dram_parameter("reconT", [GEO, NPC], F32, isOutput=True)

    with nc.allow_low_precision(reason="fp32r matmul inputs"), \
         TileContext(nc) as tc:
        with (
            tc.tile_pool(name="const", bufs=1) as cpool,
            tc.tile_pool(name="wts", bufs=1) as wpool,
            tc.tile_pool(name="persist", bufs=1) as ppool,
        ):
            # ---- constants ----
            iota_t = cpool.tile([128, 128], F32)
            nc.sync.dma_start(out=iota_t[:], in_=iota_d[:])
            ident_t = cpool.tile([128, 128], F32)
            make_identity(nc, ident_t[:])
            jloc_t = cpool.tile([128, n_tiles], F32)
            nc.sync.dma_start(out=jloc_t[:], in_=jloc_d[:])
            norm_t = cpool.tile([128, WPC], F32)
            nc.sync.dma_start(out=norm_t[:], in_=norm_d[:])
            ones_f = cpool.tile([128, 1], F32)
            nc.vector.memset(ones_f[:], 1.0)
            ones_r = cpool.tile([128, 1], F32R)
            nc.vector.tensor_copy(out=ones_r[:], in_=ones_f[:])
            eps_t = cpool.tile([1, 1], F32)
            nc.vector.memset(eps_t[:], LN_EPS)
            onesrow_f = cpool.tile([1, 128], F32)
            nc.vector.memset(onesrow_f[:], 1.0)
            onesrow_r = cpool.tile([1, 128], F32R)
            nc.vector.tensor_copy(out=onesrow_r[:], in_=onesrow_f[:])


            # ---- weights to SBUF ----
            def load_w(dram, rows, cols):
                tiles = []
                r0 = 0
                while r0 < rows:
                    r = min(128, rows - r0)
                    t = wpool.tile([r, cols], F32R, tag=f"w{dram.name}{r0}")
                    nc.sync.dma_start(out=t[:], in_=dram[r0:r0 + r, :])
                    tiles.append((t, r))
                    r0 += r
                return tiles

            wt0x = load_w(wt0x_d, GEO, 256)
            a0 = load_w(a0_d, 25, 256)
            wt1 = load_w(wt1_d, 256, 512)
            a1 = load_w(a1_d, 25, 512)
            wtnp = load_w(wtnp_d, 512, NSC)
            anp = load_w(anp_d, 25, NSC)
            wte = load_w(wte_d, 512, 512)
            wtd0 = load_w(wtd0_d, 512, 256)
            ad0 = load_w(ad0_d, 25, 256)
            wtd1 = load_w(wtd1_d, 256, GEO)
            ad1 = load_w(ad1_d, 25, GEO)

            # ================= phase A =================
            with (
                tc.tile_pool(name="msgp", bufs=3) as mpool,
                tc.tile_pool(name="subs", bufs=3) as spool,
                tc.tile_pool(name="aggp", bufs=4) as apool,
                tc.tile_pool(name="psA", bufs=4, space="PSUM") as psA,
                tc.tile_pool(name="ahp", bufs=1) as ahpool,
            ):
                ah_t = ahpool.tile([128, WPC * MSGD], F32)
                ahn_t = ahpool.tile([128, WPC * (MSGD + 1)], F32)
                for ck in range(n_chunks_a):
                    msg_t = mpool.tile([128, tiles_pc * 8 * MSGD], BF16)
                    nc.sync.dma_start(
                        out=msg_t[:],
                        in_=msg_d[:, ck * tiles_pc * 8 * MSGD:
                                  (ck + 1) * tiles_pc * 8 * MSGD])
                    subs_t = spool.tile([128, tiles_pc * MSGD], F32R)
                    msg_v = msg_t[:].rearrange("p (t s f) -> p t f s",
                                               t=tiles_pc, s=8, f=MSGD)
                    nc.vector.reduce_sum(
                        out=subs_t[:].rearrange("p (t f) -> p t f",
                                                t=tiles_pc, f=MSGD),
                        in_=msg_v, axis=mybir.AxisListType.X)
                    win_p = psA.tile([128, CW * MSGD], F32, space="PSUM",
                                     tag="win")
                    for tt in range(tiles_pc):
                        w_in_c = tt // T_W
                        k_in_w = tt % T_W
                        agg_t = apool.tile([128, 128], F32R, tag="agg")
                        gtile = ck * tiles_pc + tt
                        nc.vector.tensor_scalar(
                            out=agg_t[:], in0=iota_t[:],
                            scalar1=jloc_t[:, gtile:gtile + 1], scalar2=None,
                            op0=mybir.AluOpType.is_equal)
                        nc.tensor.matmul(
                            out=win_p[:, w_in_c * MSGD:(w_in_c + 1) * MSGD],
                            lhsT=agg_t[:],
                            rhs=subs_t[:, tt * MSGD:(tt + 1) * MSGD],
                            start=(k_in_w == 0), stop=(k_in_w == T_W - 1))
                    nc.vector.tensor_copy(
                        out=ah_t[:, ck * CW * MSGD:(ck + 1) * CW * MSGD],
                        in_=win_p[:])

                # ---- ah * norm, ones column -> ahn [128, WPC*25] ----
                ahn_v = ahn_t[:].rearrange("p (w f) -> p w f", w=WPC,
                                           f=MSGD + 1)
                nc.vector.memset(ahn_v[:, :, 0:1], 1.0)
                normB_t = mpool.tile([128, WPC * MSGD], F32, tag="msg_t")
                nc.vector.tensor_copy(
                    out=normB_t[:].rearrange("p (w f) -> p w f", w=WPC, f=MSGD),
                    in_=norm_t[:].to_broadcast([128, WPC, MSGD]))
                nc.vector.tensor_tensor(
                    out=ahn_v[:, :, 1:MSGD + 1],
                    in0=ah_t[:].rearrange("p (w f) -> p w f", w=WPC, f=MSGD),
                    in1=normB_t[:].rearrange("p (w f) -> p w f", w=WPC, f=MSGD),
                    op=mybir.AluOpType.mult)

                # ---- transpose ahn -> ahnT [25, NPC] ----
                for g in range(0, WPC, 4):
                    gw = min(4, WPC - g)
                    tr_p = psA.tile([MSGD + 1, 512], F32, space="PSUM",
                                    tag="trp")
                    for k in range(gw):
                        nc.tensor.transpose(
                            out=tr_p[:, k * 128:(k + 1) * 128],
                            in_=ahn_v[:, g + k, :], identity=ident_t[:])
                    ncols = min(NPC - g * 128, gw * 128)
                    stg = spool.tile([MSGD + 1, 512], F32R, tag="stg")
                    nc.vector.tensor_copy(out=stg[:, 0:ncols],
                                          in_=tr_p[:, 0:ncols])
                    nc.sync.dma_start(
                        out=ahnT_d[:, g * 128:g * 128 + ncols],
                        in_=stg[:, 0:ncols])

            # ================= phase B =================
            with (
                tc.tile_pool(name="actp", bufs=2) as hpool,
                tc.tile_pool(name="psB", bufs=5, space="PSUM") as psB,
                tc.tile_pool(name="psS", bufs=1, space="PSUM") as psS,
            ):
                def layer(rhs_tiles, wts, fout, cn, ln_relu, out_dtype=F32R,
                          tag=""):
                    n_mt = (fout + 127) // 128
                    zs = []
                    for mt in range(n_mt):
                        m = min(128, fout - mt * 128)
                        z_p = psB.tile([128, NODE_CHUNK], F32, space="PSUM",
                                       tag="z")
                        for kt, ((rhs_ap, k), wt_ap) in enumerate(
                                zip(rhs_tiles, wts)):
                            nc.tensor.matmul(
                                out=z_p[:m, 0:cn],
                                lhsT=wt_ap[:, mt * 128:mt * 128 + m],
                                rhs=rhs_ap,
                                start=(kt == 0),
                                stop=(kt == len(rhs_tiles) - 1))
                        zs.append((z_p, m))
                    if not ln_relu:
                        outs = []
                        for mt, (z_p, m) in enumerate(zs):
                            o = hpool.tile([128, NODE_CHUNK], out_dtype,
                                           tag=f"o{tag}{mt}")
                            nc.scalar.copy(out=o[:m, 0:cn], in_=z_p[:m, 0:cn])
                            outs.append((o, m))
                        return outs
                    var_p = psS.tile([1, NODE_CHUNK], F32, space="PSUM",
                                     tag="v")
                    zsqs = []
                    for mt, (z_p, m) in enumerate(zs):
                        zsq = hpool.tile([128, NODE_CHUNK], F32R, tag="q")
                        nc.scalar.square(out=zsq[:m, 0:cn], in_=z_p[:m, 0:cn])
                        zsqs.append((zsq, m))
                    for mt, (zsq, m) in enumerate(zsqs):
                        nc.tensor.matmul(
                            out=var_p[:, 0:cn], lhsT=ones_r[:m, :],
                            rhs=zsq[:m, 0:cn],
                            start=(mt == 0), stop=(mt == len(zsqs) - 1))
                    s_t = hpool.tile([1, NODE_CHUNK], F32R, tag="s")
                    nc.scalar.activation(
                        out=s_t[:, 0:cn], in_=var_p[:, 0:cn],
                        func=mybir.ActivationFunctionType.Sqrt,
                        bias=eps_t[:1, :1], scale=1.0 / fout)
                    sB_p = psS.tile([128, NODE_CHUNK], F32, space="PSUM",
                                    tag="sb")
                    nc.tensor.matmul(out=sB_p[:, 0:cn], lhsT=onesrow_r[:, :],
                                     rhs=s_t[:, 0:cn], start=True, stop=True)
                    rstdB = hpool.tile([128, NODE_CHUNK], F32, tag="r")
                    nc.vector.reciprocal(out=rstdB[:, 0:cn], in_=sB_p[:, 0:cn])
                    outs = []
                    for mt, (z_p, m) in enumerate(zs):
                        t_t = hpool.tile([128, NODE_CHUNK], F32, tag="t")
                        nc.vector.tensor_tensor(
                            out=t_t[:m, 0:cn], in0=z_p[:m, 0:cn],
                            in1=rstdB[:m, 0:cn],
                            op=mybir.AluOpType.mult)
                        o = hpool.tile([128, NODE_CHUNK], out_dtype,
                                       tag=f"o{tag}{mt}")
                        if ln_relu == "relu":
                            nc.scalar.activation(
                                out=o[:m, 0:cn], in_=t_t[:m, 0:cn],
                                func=mybir.ActivationFunctionType.Relu)
                        else:
                            nc.scalar.copy(out=o[:m, 0:cn], in_=t_t[:m, 0:cn])
                        outs.append((o, m))
                    return outs

                n_chunks_b = (NPC + NODE_CHUNK - 1) // NODE_CHUNK
                for nck in range(n_chunks_b):
                    c0 = nck * NODE_CHUNK
                    cn = min(NODE_CHUNK, NPC - c0)
                    sl = slice(c0, c0 + cn)
                    ahn_c = hpool.tile([MSGD + 1, NODE_CHUNK], F32R, tag="ahc")
                    nc.sync.dma_start(out=ahn_c[:, 0:cn], in_=ahnT_d[:, sl])
                    xhat_c = hpool.tile([GEO, NODE_CHUNK], F32R, tag="xhc")
                    nc.sync.dma_start(out=xhat_c[:, 0:cn], in_=xhat_d[:, sl])
                    ah25 = ahn_c[0:MSGD + 1, 0:cn]

                    h1 = layer([(xhat_c[:, 0:cn], GEO), (ah25, MSGD + 1)],
                               [wt0x[0][0], a0[0][0]],
                               256, cn, "relu", tag="e0")
                    h2 = layer([(h1[0][0][:, 0:cn], 128),
                                (h1[1][0][:, 0:cn], 128),
                                (ah25, MSGD + 1)],
                               [wt1[0][0], wt1[1][0], a1[0][0]],
                               512, cn, "relu", tag="e1")
                    h2r = [(t[:, 0:cn], 128) for t, m in h2]
                    sc = layer(h2r + [(ah25, MSGD + 1)],
                               [wtnp[0][0], wtnp[1][0], wtnp[2][0],
                                wtnp[3][0], anp[0][0]],
                               NSC, cn, "ln", out_dtype=F32, tag="np")
                    nc.sync.dma_start(out=scores_d[:, sl],
                                      in_=sc[0][0][0:NSC, 0:cn])
                    rep = layer(h2r,
                                [wte[0][0], wte[1][0], wte[2][0], wte[3][0]],
                                512, cn, False, tag="ed")
                    repr_ = [(t[:, 0:cn], 128) for t, m in rep]
                    d1 = layer(repr_ + [(ah25, MSGD + 1)],
                               [wtd0[0][0], wtd0[1][0], wtd0[2][0],
                                wtd0[3][0], ad0[0][0]],
                               256, cn, "relu", tag="d0")
                    rec = layer([(d1[0][0][:, 0:cn], 128),
                                 (d1[1][0][:, 0:cn], 128),
                                 (ah25, MSGD + 1)],
                                [wtd1[0][0], wtd1[1][0], ad1[0][0]],
                                GEO, cn, "relu", out_dtype=F32, tag="d1")
                    nc.sync.dma_start(out=recon_d[:, sl],
                                      in_=rec[0][0][0:GEO, 0:cn])

    nc.compile()
    return nc


# --------------------------------------------------------------------------
# entry point
# --------------------------------------------------------------------------

def kernel(**inputs):
    import os
    per_core, w, meta = _prepare(inputs)
    T_W = meta["T_W"]
    if T_W not in _PROG_CACHE:
        _PROG_CACHE[T_W] = _build_program(T_W)
    nc = _PROG_CACHE[T_W]

    in_maps = []
    for c in range(NCORES):
        m = dict(per_core[c])
        m.update(w)
        in_maps.append(m)

    import time as _time
    trace = bool(int(os.environ.get("BASS_KERNEL_TRACE", "0")))
    t0 = _time.time()
    try:
        res = run_bass_kernel_spmd(nc, in_maps, list(range(NCORES)), trace=trace)
    except ModuleNotFoundError:
        res = run_bass_kernel_spmd(nc, in_maps, list(range(NCORES)), trace=False)
    exec_wall = _time.time() - t0
    if getattr(res, "exec_time_ns", None) is not None:
        print(f"HW exec time: {res.exec_time_ns} ns")
    else:
        print(f"HW exec time: {int(exec_wall * 1e9)} ns (wall-clock of run, no profile)")

    scores = np.concatenate([res.results[c]["scoresT"].T for c in range(NCORES)],
                            axis=0)
    recon = np.concatenate([res.results[c]["reconT"].T for c in range(NCORES)],
                           axis=0)
    g = np.asarray(inputs["np_g"], np.float32)
    beta = np.asarray(inputs["np_beta"], np.float32)
    n_scores = scores * g[None, :] + beta[None, :]
    mask_nodes = np.asarray(inputs["mask_nodes"], np.int64)
    x = np.asarray(inputs["x"], np.float32)
    x_pred = recon[mask_nodes].astype(np.float32)
    x_true = x[mask_nodes].astype(np.float32)
    return (x_pred, x_true, n_scores.astype(np.float32))


# revision 9
# speedup vs baseline: 1.8617x; 1.0145x over previous
"""Trainium2 Bass kernel for the masked-autoencoder SAGE GNN problem.

Strategy (8 NeuronCores, node-sharded by dst range):
- Host (numpy, index/layout glue): sort edges by dst, bucket to 8 cores
  (12500 nodes each), lay per-edge messages out into a padded
  [128 part, tiles, 8 slots, 24 feat] bf16 stream per core (pad-to-8 per
  node); bf16 halves the host->device transfer, the dominant wall cost.
- Device per core: segment-sum via (a) 8->1 slot reduce on DVE,
  (b) per-128-sub one-hot aggregation matmuls on PE accumulating whole
  128-node windows in PSUM; then norm scaling, PE transpose to
  feature-major, and the full 6-matmul MLP pipeline (centered-weight
  LayerNorm trick: mean removed exactly inside the matmul; var via
  ACT square + ones-matmul; rstd broadcast via PE ones outer product).
- Outputs per core: n_scores_raw.T [16, 12500] and recon.T [10, 12500];
  host applies the (g, beta) affine of the final LayerNorm and the
  mask_nodes row-gather.
"""

import sys

sys.path.insert(0, "/opt/trn_rl_repo")

import numpy as np

import concourse.bass as bass
import concourse.bacc as bacc
import concourse.mybir as mybir
from concourse.tile import TileContext
from concourse.bass_utils import run_bass_kernel_spmd
from concourse.masks import make_identity

F32 = mybir.dt.float32
F32R = mybir.dt.float32r
BF16 = mybir.dt.bfloat16

N = 100000
E = 3200000
GEO = 10
EFD = 14          # edge-feature dim: dist(1) angle(1) feat(8) disc(4)
MSGD = 24         # GEO + EFD
NCORES = 8
NPC = N // NCORES         # 12500 nodes per core
WPC = (NPC + 127) // 128  # 98 windows of 128 nodes
NPAD = WPC * 128          # 12544
LN_EPS = 1e-5
DIMS = (10, 256, 512)
NSC = 16                  # node classes
CHUNK_W = 2               # windows per phase-A chunk
NODE_CHUNK = 512          # nodes per phase-B chunk

_PROG_CACHE = {}


# --------------------------------------------------------------------------
# host-side preparation
# --------------------------------------------------------------------------

def _center(W, b):
    """Fold LayerNorm mean-subtraction into weights: rows of result produce
    z - mean_over_rows(z) exactly."""
    Wc = W - W.mean(axis=0, keepdims=True)
    bc = b - b.mean()
    return Wc.astype(np.float32), bc.astype(np.float32)


def _prepare(inputs):
    x = np.asarray(inputs["x"], np.float32)
    distance = np.asarray(inputs["distance"], np.float32)
    angle = np.asarray(inputs["angle"], np.float32)
    feat = np.asarray(inputs["feat"], np.float32)
    disc = np.asarray(inputs["disc"], np.float32)
    norm = np.asarray(inputs["norm"], np.float32)
    src = np.asarray(inputs["src"], np.int64)
    dst = np.asarray(inputs["dst"], np.int64)
    mask_nodes = np.asarray(inputs["mask_nodes"], np.int64)
    token = np.asarray(inputs["enc_mask_token"], np.float32)

    order = np.argsort(dst, kind="stable")
    dst_s = dst[order]
    bounds = np.searchsorted(dst_s, np.arange(NCORES + 1) * NPC)
    deg_all = np.bincount(dst, minlength=N).astype(np.int64)
    csub_all = (deg_all + 7) // 8             # subs per node (0 for deg 0)

    # windows: per core, per 128-node window
    csub_pad = np.zeros(NCORES * NPAD, np.int64)
    csub_pad.reshape(NCORES, NPAD)[:, :NPC] = csub_all.reshape(NCORES, NPC)
    s_w = csub_pad.reshape(NCORES, WPC, 128).sum(axis=2)     # [8, 98]
    T_W = int(((s_w + 127) // 128).max())
    n_tiles = WPC * T_W

    # masked x for encoder input
    out_x = x.copy()
    out_x[mask_nodes] = token[0]

    per_core = []
    cum_all = np.concatenate([[0], np.cumsum(deg_all)])
    for c in range(NCORES):
        lo, hi = int(bounds[c]), int(bounds[c + 1])
        e_ids = order[lo:hi]
        deg = deg_all[c * NPC:(c + 1) * NPC]
        csub = csub_all[c * NPC:(c + 1) * NPC]
        S_real = int(csub.sum())
        nodes_rep = np.repeat(np.arange(NPC), csub)          # node of each sub
        # k index of sub within its node
        sub_start = np.concatenate([[0], np.cumsum(csub)])
        sub_k = np.arange(S_real) - sub_start[nodes_rep]
        win_of_sub = nodes_rep // 128
        # rank of sub within window
        wsum = csub_pad.reshape(NCORES, WPC, 128)[c].sum(axis=1)
        win_start = np.concatenate([[0], np.cumsum(wsum)])
        pos_in_win = np.arange(S_real) - win_start[win_of_sub]
        t_in_w = pos_in_win // 128
        p_of_sub = (pos_in_win % 128).astype(np.int64)
        tile_of_sub = win_of_sub * T_W + t_in_w

        jloc = np.full((128, n_tiles), -1.0, np.float32)
        jloc[p_of_sub, tile_of_sub] = (nodes_rep % 128).astype(np.float32)

        # slot-level edge assignment
        sub_rep = np.repeat(np.arange(S_real), 8)
        j2 = np.tile(np.arange(8), S_real)
        node_rep8 = nodes_rep[sub_rep]
        e_pos = deg.cumsum()[node_rep8] - deg[node_rep8] + sub_k[sub_rep] * 8 + j2
        valid = e_pos < (deg.cumsum())[node_rep8]
        ev = e_ids[e_pos[valid]]              # global edge ids per valid slot

        msg = np.zeros((128, n_tiles, 8, MSGD), np.float32)
        pv = p_of_sub[sub_rep][valid]
        tv = tile_of_sub[sub_rep][valid]
        jv = j2[valid]
        msg[pv, tv, jv, 0:GEO] = x[src[ev]]
        msg[pv, tv, jv, GEO + 0] = distance[ev]
        msg[pv, tv, jv, GEO + 1] = angle[ev]
        msg[pv, tv, jv, GEO + 2:GEO + 10] = feat[ev]
        msg[pv, tv, jv, GEO + 10:GEO + 14] = disc[ev]

        norm_nm = np.zeros((128, WPC), np.float32)
        nc_pad = np.zeros(NPAD, np.float32)
        nc_pad[:NPC] = norm[c * NPC:(c + 1) * NPC, 0]
        norm_nm[:, :] = nc_pad.reshape(WPC, 128).T

        xhatT = out_x[c * NPC:(c + 1) * NPC].T.copy()

        import ml_dtypes
        per_core.append(dict(
            msg=msg.reshape(128, n_tiles * 8 * MSGD).astype(ml_dtypes.bfloat16),
            jloc=jloc,
            norm_nm=norm_nm,
            xhatT=xhatT,
        ))

    # ---- weights (shared across cores) ----
    w = {}
    e0W, e0b = _center(inputs["enc0_W"], inputs["enc0_b"])
    e1W, e1b = _center(inputs["enc1_W"], inputs["enc1_b"])
    d0W, d0b = _center(inputs["dec0_W"], inputs["dec0_b"])
    d1W, d1b = _center(inputs["dec1_W"], inputs["dec1_b"])
    npW, npb = _center(inputs["np_W"], inputs["np_b"])
    e2dW = np.asarray(inputs["e2d_W"], np.float32)

    # per-layer: h-part K-tiles + ah-tile rows [bias ; Wa^T] matching
    # rhs ah25 = [ones ; ah*norm] (ones row FIRST, partition 0)
    w["wt0x"] = e0W[:, 0:GEO].T.copy()                                   # [10, 256]
    w["a0"] = np.concatenate([e0b[None, :], e0W[:, GEO:34].T], axis=0)   # [25, 256]
    w["wt1"] = e1W[:, 0:256].T.copy()                                    # [256, 512]
    w["a1"] = np.concatenate([e1b[None, :], e1W[:, 256:280].T], axis=0)  # [25, 512]
    w["wtnp"] = npW.T.copy()                                             # [512, 16]
    w["anp"] = np.concatenate([npb[None, :],
                               np.zeros((MSGD, NSC), np.float32)], axis=0)
    w["wte"] = e2dW.T.copy()                                             # [512, 512]
    w["wtd0"] = d0W[:, 0:512].T.copy()                                   # [512, 256]
    w["ad0"] = np.concatenate([d0b[None, :], d0W[:, 512:536].T], axis=0)
    w["wtd1"] = d1W[:, 0:256].T.copy()                                   # [256, 10]
    w["ad1"] = np.concatenate([d1b[None, :], d1W[:, 256:280].T], axis=0)
    iota = np.tile(np.arange(128, dtype=np.float32), (128, 1))
    w["iota"] = iota

    meta = dict(T_W=T_W, n_tiles=n_tiles)
    return per_core, w, meta


# --------------------------------------------------------------------------
# device program
# --------------------------------------------------------------------------

def _build_program(T_W):
    n_tiles = WPC * T_W
    CW = CHUNK_W
    n_chunks_a = WPC // CW                      # 49
    tiles_pc = CW * T_W                         # tiles per chunk
    msg_w = n_tiles * 8 * MSGD                  # msg cols per partition

    nc = bacc.Bacc("TRN2", target_bir_lowering=False)

    msg_d = nc.declare_dram_parameter("msg", [128, msg_w], BF16, isOutput=False)
    jloc_d = nc.declare_dram_parameter("jloc", [128, n_tiles], F32, isOutput=False)
    iota_d = nc.declare_dram_parameter("iota", [128, 128], F32, isOutput=False)
    norm_d = nc.declare_dram_parameter("norm_nm", [128, WPC], F32, isOutput=False)
    xhat_d = nc.declare_dram_parameter("xhatT", [GEO, NPC], F32R, isOutput=False)
    wt0x_d = nc.declare_dram_parameter("wt0x", [GEO, 256], F32R, isOutput=False)
    a0_d = nc.declare_dram_parameter("a0", [25, 256], F32R, isOutput=False)
    wt1_d = nc.declare_dram_parameter("wt1", [256, 512], F32R, isOutput=False)
    a1_d = nc.declare_dram_parameter("a1", [25, 512], F32R, isOutput=False)
    wtnp_d = nc.declare_dram_parameter("wtnp", [512, NSC], F32R, isOutput=False)
    anp_d = nc.declare_dram_parameter("anp", [25, NSC], F32R, isOutput=False)
    wte_d = nc.declare_dram_parameter("wte", [512, 512], F32R, isOutput=False)
    wtd0_d = nc.declare_dram_parameter("wtd0", [512, 256], F32R, isOutput=False)
    ad0_d = nc.declare_dram_parameter("ad0", [25, 256], F32R, isOutput=False)
    wtd1_d = nc.declare_dram_parameter("wtd1", [256, GEO], F32R, isOutput=False)
    ad1_d = nc.declare_dram_parameter("ad1", [25, GEO], F32R, isOutput=False)
    scores_d = nc.declare_dram_parameter("scoresT", [NSC, NPC], F32, isOutput=True)
    ahnT_d = nc.dram_tensor("ahnT_buf", [MSGD + 1, NPC], F32R)
    recon_d = nc.declare_dram_parameter("reconT", [GEO, NPC], F32, isOutput=True)

    with nc.allow_low_precision(reason="fp32r matmul inputs"), \
         TileContext(nc) as tc:
        with (
            tc.tile_pool(name="const", bufs=1) as cpool,
            tc.tile_pool(name="wts", bufs=1) as wpool,
            tc.tile_pool(name="persist", bufs=1) as ppool,
        ):
            # ---- constants ----
            iota_t = cpool.tile([128, 128], F32)
            nc.sync.dma_start(out=iota_t[:], in_=iota_d[:])
            ident_t = cpool.tile([128, 128], F32)
            make_identity(nc, ident_t[:])
            jloc_t = cpool.tile([128, n_tiles], F32)
            nc.sync.dma_start(out=jloc_t[:], in_=jloc_d[:])
            norm_t = cpool.tile([128, WPC], F32)
            nc.sync.dma_start(out=norm_t[:], in_=norm_d[:])
            ones_f = cpool.tile([128, 1], F32)
            nc.vector.memset(ones_f[:], 1.0)
            ones_r = cpool.tile([128, 1], F32R)
            nc.vector.tensor_copy(out=ones_r[:], in_=ones_f[:])
            eps_t = cpool.tile([1, 1], F32)
            nc.vector.memset(eps_t[:], LN_EPS)
            onesrow_f = cpool.tile([1, 128], F32)
            nc.vector.memset(onesrow_f[:], 1.0)
            onesrow_r = cpool.tile([1, 128], F32R)
            nc.vector.tensor_copy(out=onesrow_r[:], in_=onesrow_f[:])


            # ---- weights to SBUF ----
            def load_w(dram, rows, cols):
                tiles = []
                r0 = 0
                while r0 < rows:
                    r = min(128, rows - r0)
                    t = wpool.tile([r, cols], F32R, tag=f"w{dram.name}{r0}")
                    nc.sync.dma_start(out=t[:], in_=dram[r0:r0 + r, :])
                    tiles.append((t, r))
                    r0 += r
                return tiles

            wt0x = load_w(wt0x_d, GEO, 256)
            a0 = load_w(a0_d, 25, 256)
            wt1 = load_w(wt1_d, 256, 512)
            a1 = load_w(a1_d, 25, 512)
            wtnp = load_w(wtnp_d, 512, NSC)
            anp = load_w(anp_d, 25, NSC)
            wte = load_w(wte_d, 512, 512)
            wtd0 = load_w(wtd0_d, 512, 256)
            ad0 = load_w(ad0_d, 25, 256)
            wtd1 = load_w(wtd1_d, 256, GEO)
            ad1 = load_w(ad1_d, 25, GEO)

            # ================= phase A =================
            with (
                tc.tile_pool(name="msgp", bufs=3) as mpool,
                tc.tile_pool(name="subs", bufs=3) as spool,
                tc.tile_pool(name="aggp", bufs=4) as apool,
                tc.tile_pool(name="psA", bufs=4, space="PSUM") as psA,
                tc.tile_pool(name="ahp", bufs=1) as ahpool,
            ):
                ah_t = ahpool.tile([128, WPC * MSGD], F32)
                ahn_t = ahpool.tile([128, WPC * (MSGD + 1)], F32)
                for ck in range(n_chunks_a):
                    msg_t = mpool.tile([128, tiles_pc * 8 * MSGD], BF16)
                    nc.sync.dma_start(
                        out=msg_t[:],
                        in_=msg_d[:, ck * tiles_pc * 8 * MSGD:
                                  (ck + 1) * tiles_pc * 8 * MSGD])
                    subs_t = spool.tile([128, tiles_pc * MSGD], F32R)
                    msg_v = msg_t[:].rearrange("p (t s f) -> p t f s",
                                               t=tiles_pc, s=8, f=MSGD)
                    nc.vector.reduce_sum(
                        out=subs_t[:].rearrange("p (t f) -> p t f",
                                                t=tiles_pc, f=MSGD),
                        in_=msg_v, axis=mybir.AxisListType.X)
                    win_p = psA.tile([128, CW * MSGD], F32, space="PSUM",
                                     tag="win")
                    for tt in range(tiles_pc):
                        w_in_c = tt // T_W
                        k_in_w = tt % T_W
                        agg_t = apool.tile([128, 128], F32R, tag="agg")
                        gtile = ck * tiles_pc + tt
                        nc.vector.tensor_scalar(
                            out=agg_t[:], in0=iota_t[:],
                            scalar1=jloc_t[:, gtile:gtile + 1], scalar2=None,
                            op0=mybir.AluOpType.is_equal)
                        nc.tensor.matmul(
                            out=win_p[:, w_in_c * MSGD:(w_in_c + 1) * MSGD],
                            lhsT=agg_t[:],
                            rhs=subs_t[:, tt * MSGD:(tt + 1) * MSGD],
                            start=(k_in_w == 0), stop=(k_in_w == T_W - 1))
                    nc.vector.tensor_copy(
                        out=ah_t[:, ck * CW * MSGD:(ck + 1) * CW * MSGD],
                        in_=win_p[:])

                # ---- ah * norm, ones column -> ahn [128, WPC*25] ----
                ahn_v = ahn_t[:].rearrange("p (w f) -> p w f", w=WPC,
                                           f=MSGD + 1)
                nc.vector.memset(ahn_v[:, :, 0:1], 1.0)
                normB_t = mpool.tile([128, WPC * MSGD], F32, tag="msg_t")
                nc.vector.tensor_copy(
                    out=normB_t[:].rearrange("p (w f) -> p w f", w=WPC, f=MSGD),
                    in_=norm_t[:].to_broadcast([128, WPC, MSGD]))
                nc.vector.tensor_tensor(
                    out=ahn_v[:, :, 1:MSGD + 1],
                    in0=ah_t[:].rearrange("p (w f) -> p w f", w=WPC, f=MSGD),
                    in1=normB_t[:].rearrange("p (w f) -> p w f", w=WPC, f=MSGD),
                    op=mybir.AluOpType.mult)

                # ---- transpose ahn -> ahnT [25, NPC] ----
                for g in range(0, WPC, 4):
                    gw = min(4, WPC - g)
                    tr_p = psA.tile([MSGD + 1, 512], F32, space="PSUM",
                                    tag="trp")
                    for k in range(gw):
                        nc.tensor.transpose(
                            out=tr_p[:, k * 128:(k + 1) * 128],
                            in_=ahn_v[:, g + k, :], identity=ident_t[:])
                    ncols = min(NPC - g * 128, gw * 128)
                    stg = spool.tile([MSGD + 1, 512], F32R, tag="stg")
                    nc.vector.tensor_copy(out=stg[:, 0:ncols],
                                          in_=tr_p[:, 0:ncols])
                    nc.sync.dma_start(
                        out=ahnT_d[:, g * 128:g * 128 + ncols],
                        in_=stg[:, 0:ncols])

            # ================= phase B =================
            with (
                tc.tile_pool(name="actp", bufs=2) as hpool,
                tc.tile_pool(name="psB", bufs=5, space="PSUM") as psB,
                tc.tile_pool(name="psS", bufs=1, space="PSUM") as psS,
            ):
                def layer(rhs_tiles, wts, fout, cn, ln_relu, out_dtype=F32R,
                          tag=""):
                    n_mt = (fout + 127) // 128
                    zs = []
                    for mt in range(n_mt):
                        m = min(128, fout - mt * 128)
                        z_p = psB.tile([128, NODE_CHUNK], F32, space="PSUM",
                                       tag="z")
                        for kt, ((rhs_ap, k), wt_ap) in enumerate(
                                zip(rhs_tiles, wts)):
                            nc.tensor.matmul(
                                out=z_p[:m, 0:cn],
                                lhsT=wt_ap[:, mt * 128:mt * 128 + m],
                                rhs=rhs_ap,
                                start=(kt == 0),
                                stop=(kt == len(rhs_tiles) - 1))
                        zs.append((z_p, m))
                    if not ln_relu:
                        outs = []
                        for mt, (z_p, m) in enumerate(zs):
                            o = hpool.tile([128, NODE_CHUNK], out_dtype,
                                           tag=f"o{tag}{mt}")
                            nc.scalar.copy(out=o[:m, 0:cn], in_=z_p[:m, 0:cn])
                            outs.append((o, m))
                        return outs
                    var_p = psS.tile([1, NODE_CHUNK], F32, space="PSUM",
                                     tag="v")
                    zsqs = []
                    for mt, (z_p, m) in enumerate(zs):
                        zsq = hpool.tile([128, NODE_CHUNK], F32R, tag="q")
                        nc.scalar.square(out=zsq[:m, 0:cn], in_=z_p[:m, 0:cn])
                        zsqs.append((zsq, m))
                    for mt, (zsq, m) in enumerate(zsqs):
                        nc.tensor.matmul(
                            out=var_p[:, 0:cn], lhsT=ones_r[:m, :],
                            rhs=zsq[:m, 0:cn],
                            start=(mt == 0), stop=(mt == len(zsqs) - 1))
                    s_t = hpool.tile([1, NODE_CHUNK], F32R, tag="s")
                    nc.scalar.activation(
                        out=s_t[:, 0:cn], in_=var_p[:, 0:cn],
                        func=mybir.ActivationFunctionType.Sqrt,
                        bias=eps_t[:1, :1], scale=1.0 / fout)
                    sB_p = psS.tile([128, NODE_CHUNK], F32, space="PSUM",
                                    tag="sb")
                    nc.tensor.matmul(out=sB_p[:, 0:cn], lhsT=onesrow_r[:, :],
                                     rhs=s_t[:, 0:cn], start=True, stop=True)
                    rstdB = hpool.tile([128, NODE_CHUNK], F32, tag="r")
                    nc.vector.reciprocal(out=rstdB[:, 0:cn], in_=sB_p[:, 0:cn])
                    outs = []
                    for mt, (z_p, m) in enumerate(zs):
                        t_t = hpool.tile([128, NODE_CHUNK], F32, tag="t")
                        nc.vector.tensor_tensor(
                            out=t_t[:m, 0:cn], in0=z_p[:m, 0:cn],
                            in1=rstdB[:m, 0:cn],
                            op=mybir.AluOpType.mult)
                        o = hpool.tile([128, NODE_CHUNK], out_dtype,
                                       tag=f"o{tag}{mt}")
                        if ln_relu == "relu":
                            nc.scalar.activation(
                                out=o[:m, 0:cn], in_=t_t[:m, 0:cn],
                                func=mybir.ActivationFunctionType.Relu)
                        else:
                            nc.scalar.copy(out=o[:m, 0:cn], in_=t_t[:m, 0:cn])
                        outs.append((o, m))
                    return outs

                n_chunks_b = (NPC + NODE_CHUNK - 1) // NODE_CHUNK
                for nck in range(n_chunks_b):
                    c0 = nck * NODE_CHUNK
                    cn = min(NODE_CHUNK, NPC - c0)
                    sl = slice(c0, c0 + cn)
                    ahn_c = hpool.tile([MSGD + 1, NODE_CHUNK], F32R, tag="ahc")
                    nc.sync.dma_start(out=ahn_c[:, 0:cn], in_=ahnT_d[:, sl])
                    xhat_c = hpool.tile([GEO, NODE_CHUNK], F32R, tag="xhc")
                    nc.sync.dma_start(out=xhat_c[:, 0:cn], in_=xhat_d[:, sl])
                    ah25 = ahn_c[0:MSGD + 1, 0:cn]

                    h1 = layer([(xhat_c[:, 0:cn], GEO), (ah25, MSGD + 1)],
                               [wt0x[0][0], a0[0][0]],
                               256, cn, "relu", tag="e0")
                    h2 = layer([(h1[0][0][:, 0:cn], 128),
                                (h1[1][0][:, 0:cn], 128),
                                (ah25, MSGD + 1)],
                               [wt1[0][0], wt1[1][0], a1[0][0]],
                               512, cn, "relu", tag="e1")
                    h2r = [(t[:, 0:cn], 128) for t, m in h2]
                    sc = layer(h2r + [(ah25, MSGD + 1)],
                               [wtnp[0][0], wtnp[1][0], wtnp[2][0],
                                wtnp[3][0], anp[0][0]],
                               NSC, cn, "ln", out_dtype=F32, tag="np")
                    nc.sync.dma_start(out=scores_d[:, sl],
                                      in_=sc[0][0][0:NSC, 0:cn])
                    rep = layer(h2r,
                                [wte[0][0], wte[1][0], wte[2][0], wte[3][0]],
                                512, cn, False, tag="ed")
                    repr_ = [(t[:, 0:cn], 128) for t, m in rep]
                    d1 = layer(repr_ + [(ah25, MSGD + 1)],
                               [wtd0[0][0], wtd0[1][0], wtd0[2][0],
                                wtd0[3][0], ad0[0][0]],
                               256, cn, "relu", tag="d0")
                    rec = layer([(d1[0][0][:, 0:cn], 128),
                                 (d1[1][0][:, 0:cn], 128),
                                 (ah25, MSGD + 1)],
                                [wtd1[0][0], wtd1[1][0], ad1[0][0]],
                                GEO, cn, "relu", out_dtype=F32, tag="d1")
                    nc.sync.dma_start(out=recon_d[:, sl],
                                      in_=rec[0][0][0:GEO, 0:cn])

    nc.compile()
    return nc


# --------------------------------------------------------------------------
# entry point
# --------------------------------------------------------------------------

def kernel(**inputs):
    import os
    per_core, w, meta = _prepare(inputs)
    T_W = meta["T_W"]
    if T_W not in _PROG_CACHE:
        _PROG_CACHE[T_W] = _build_program(T_W)
    nc = _PROG_CACHE[T_W]

    in_maps = []
    for c in range(NCORES):
        m = dict(per_core[c])
        m.update(w)
        in_maps.append(m)

    import time as _time
    trace = bool(int(os.environ.get("BASS_KERNEL_TRACE", "0")))
    t0 = _time.time()
    try:
        res = run_bass_kernel_spmd(nc, in_maps, list(range(NCORES)), trace=trace)
    except ModuleNotFoundError:
        res = run_bass_kernel_spmd(nc, in_maps, list(range(NCORES)), trace=False)
    exec_wall = _time.time() - t0
    if getattr(res, "exec_time_ns", None) is not None:
        print(f"HW exec time: {res.exec_time_ns} ns")
    else:
        print(f"HW exec time: {int(exec_wall * 1e9)} ns (wall-clock of run, no profile)")

    scores = np.concatenate([res.results[c]["scoresT"].T for c in range(NCORES)],
                            axis=0)
    recon = np.concatenate([res.results[c]["reconT"].T for c in range(NCORES)],
                           axis=0)
    g = np.asarray(inputs["np_g"], np.float32)
    beta = np.asarray(inputs["np_beta"], np.float32)
    n_scores = scores * g[None, :] + beta[None, :]
    mask_nodes = np.asarray(inputs["mask_nodes"], np.int64)
    x = np.asarray(inputs["x"], np.float32)
    x_pred = recon[mask_nodes].astype(np.float32)
    x_true = x[mask_nodes].astype(np.float32)
    return (x_pred, x_true, n_scores.astype(np.float32))


# revision 10
# speedup vs baseline: 1.8834x; 1.0117x over previous
"""Trainium2 Bass kernel for the masked-autoencoder SAGE GNN problem.

Strategy (8 NeuronCores, node-sharded by dst range):
- Host (numpy, index/layout glue): sort edges by dst, bucket to 8 cores
  (12500 nodes each), lay per-edge messages out into a padded
  [128 part, tiles, 8 slots, 24 feat] bf16 stream per core (pad-to-8 per
  node); bf16 halves the host->device transfer, the dominant wall cost.
- Device per core: segment-sum via (a) 8->1 slot reduce on DVE,
  (b) per-128-sub one-hot aggregation matmuls on PE accumulating whole
  128-node windows in PSUM; then norm scaling, PE transpose to
  feature-major, and the full 6-matmul MLP pipeline (centered-weight
  LayerNorm trick: mean removed exactly inside the matmul; var via
  ACT square + ones-matmul; rstd broadcast via PE ones outer product).
- Outputs per core: n_scores_raw.T [16, 12500] and recon.T [10, 12500];
  host applies the (g, beta) affine of the final LayerNorm and the
  mask_nodes row-gather.
"""

import sys

sys.path.insert(0, "/opt/trn_rl_repo")

import numpy as np
import ml_dtypes

import concourse.bass as bass
import concourse.bacc as bacc
import concourse.mybir as mybir
from concourse.tile import TileContext
from concourse.bass_utils import run_bass_kernel_spmd
from concourse.masks import make_identity

F32 = mybir.dt.float32
F32R = mybir.dt.float32r
BF16 = mybir.dt.bfloat16

N = 100000
E = 3200000
GEO = 10
EFD = 14          # edge-feature dim: dist(1) angle(1) feat(8) disc(4)
MSGD = 24         # GEO + EFD
NCORES = 8
NPC = N // NCORES         # 12500 nodes per core
WPC = (NPC + 127) // 128  # 98 windows of 128 nodes
NPAD = WPC * 128          # 12544
LN_EPS = 1e-5
DIMS = (10, 256, 512)
NSC = 16                  # node classes
CHUNK_W = 2               # windows per phase-A chunk
NODE_CHUNK = 512          # nodes per phase-B chunk

_PROG_CACHE = {}


# --------------------------------------------------------------------------
# host-side preparation
# --------------------------------------------------------------------------

def _center(W, b):
    """Fold LayerNorm mean-subtraction into weights: rows of result produce
    z - mean_over_rows(z) exactly."""
    Wc = W - W.mean(axis=0, keepdims=True)
    bc = b - b.mean()
    return Wc.astype(np.float32), bc.astype(np.float32)


def _prepare(inputs):
    x = np.asarray(inputs["x"], np.float32)
    distance = np.asarray(inputs["distance"], np.float32)
    angle = np.asarray(inputs["angle"], np.float32)
    feat = np.asarray(inputs["feat"], np.float32)
    disc = np.asarray(inputs["disc"], np.float32)
    norm = np.asarray(inputs["norm"], np.float32)
    src = np.asarray(inputs["src"], np.int64)
    dst = np.asarray(inputs["dst"], np.int64)
    mask_nodes = np.asarray(inputs["mask_nodes"], np.int64)
    token = np.asarray(inputs["enc_mask_token"], np.float32)

    order = np.argsort(dst, kind="stable")
    dst_s = dst[order]
    bounds = np.searchsorted(dst_s, np.arange(NCORES + 1) * NPC)
    deg_all = np.bincount(dst, minlength=N).astype(np.int64)
    csub_all = (deg_all + 7) // 8             # subs per node (0 for deg 0)

    # windows: per core, per 128-node window
    csub_pad = np.zeros(NCORES * NPAD, np.int64)
    csub_pad.reshape(NCORES, NPAD)[:, :NPC] = csub_all.reshape(NCORES, NPC)
    s_w = csub_pad.reshape(NCORES, WPC, 128).sum(axis=2)     # [8, 98]
    T_W = int(((s_w + 127) // 128).max())
    n_tiles = WPC * T_W

    # masked x for encoder input
    out_x = x.copy()
    out_x[mask_nodes] = token[0]

    per_core = []
    cum_all = np.concatenate([[0], np.cumsum(deg_all)])
    for c in range(NCORES):
        lo, hi = int(bounds[c]), int(bounds[c + 1])
        e_ids = order[lo:hi]
        deg = deg_all[c * NPC:(c + 1) * NPC]
        csub = csub_all[c * NPC:(c + 1) * NPC]
        S_real = int(csub.sum())
        nodes_rep = np.repeat(np.arange(NPC), csub)          # node of each sub
        # k index of sub within its node
        sub_start = np.concatenate([[0], np.cumsum(csub)])
        sub_k = np.arange(S_real) - sub_start[nodes_rep]
        win_of_sub = nodes_rep // 128
        # rank of sub within window
        wsum = csub_pad.reshape(NCORES, WPC, 128)[c].sum(axis=1)
        win_start = np.concatenate([[0], np.cumsum(wsum)])
        pos_in_win = np.arange(S_real) - win_start[win_of_sub]
        t_in_w = pos_in_win // 128
        p_of_sub = (pos_in_win % 128).astype(np.int64)
        tile_of_sub = win_of_sub * T_W + t_in_w

        jloc = np.full((128, n_tiles), -1.0, np.float32)
        jloc[p_of_sub, tile_of_sub] = (nodes_rep % 128).astype(np.float32)

        # slot-level edge assignment
        sub_rep = np.repeat(np.arange(S_real), 8)
        j2 = np.tile(np.arange(8), S_real)
        node_rep8 = nodes_rep[sub_rep]
        e_pos = deg.cumsum()[node_rep8] - deg[node_rep8] + sub_k[sub_rep] * 8 + j2
        valid = e_pos < (deg.cumsum())[node_rep8]
        ev = e_ids[e_pos[valid]]              # global edge ids per valid slot

        msg = np.zeros((128 * n_tiles * 8, MSGD), ml_dtypes.bfloat16)
        pv = p_of_sub[sub_rep][valid]
        tv = tile_of_sub[sub_rep][valid]
        jv = j2[valid]
        off = (pv * n_tiles + tv) * 8 + jv
        vals = np.empty((len(ev), MSGD), np.float32)
        vals[:, 0:GEO] = x[src[ev]]
        vals[:, GEO + 0] = distance[ev]
        vals[:, GEO + 1] = angle[ev]
        vals[:, GEO + 2:GEO + 10] = feat[ev]
        vals[:, GEO + 10:GEO + 14] = disc[ev]
        msg[off] = vals.astype(ml_dtypes.bfloat16)

        norm_nm = np.zeros((128, WPC), np.float32)
        nc_pad = np.zeros(NPAD, np.float32)
        nc_pad[:NPC] = norm[c * NPC:(c + 1) * NPC, 0]
        norm_nm[:, :] = nc_pad.reshape(WPC, 128).T

        xhatT = out_x[c * NPC:(c + 1) * NPC].T.copy()

        per_core.append(dict(
            msg=msg.reshape(128, n_tiles * 8 * MSGD),
            jloc=jloc,
            norm_nm=norm_nm,
            xhatT=xhatT,
        ))

    # ---- weights (shared across cores) ----
    w = {}
    e0W, e0b = _center(inputs["enc0_W"], inputs["enc0_b"])
    e1W, e1b = _center(inputs["enc1_W"], inputs["enc1_b"])
    d0W, d0b = _center(inputs["dec0_W"], inputs["dec0_b"])
    d1W, d1b = _center(inputs["dec1_W"], inputs["dec1_b"])
    npW, npb = _center(inputs["np_W"], inputs["np_b"])
    e2dW = np.asarray(inputs["e2d_W"], np.float32)

    # per-layer: h-part K-tiles + ah-tile rows [bias ; Wa^T] matching
    # rhs ah25 = [ones ; ah*norm] (ones row FIRST, partition 0)
    w["wt0x"] = e0W[:, 0:GEO].T.copy()                                   # [10, 256]
    w["a0"] = np.concatenate([e0b[None, :], e0W[:, GEO:34].T], axis=0)   # [25, 256]
    w["wt1"] = e1W[:, 0:256].T.copy()                                    # [256, 512]
    w["a1"] = np.concatenate([e1b[None, :], e1W[:, 256:280].T], axis=0)  # [25, 512]
    w["wtnp"] = npW.T.copy()                                             # [512, 16]
    w["anp"] = np.concatenate([npb[None, :],
                               np.zeros((MSGD, NSC), np.float32)], axis=0)
    w["wte"] = e2dW.T.copy()                                             # [512, 512]
    w["wtd0"] = d0W[:, 0:512].T.copy()                                   # [512, 256]
    w["ad0"] = np.concatenate([d0b[None, :], d0W[:, 512:536].T], axis=0)
    w["wtd1"] = d1W[:, 0:256].T.copy()                                   # [256, 10]
    w["ad1"] = np.concatenate([d1b[None, :], d1W[:, 256:280].T], axis=0)
    iota = np.tile(np.arange(128, dtype=np.float32), (128, 1))
    w["iota"] = iota

    meta = dict(T_W=T_W, n_tiles=n_tiles)
    return per_core, w, meta


# --------------------------------------------------------------------------
# device program
# --------------------------------------------------------------------------

def _build_program(T_W):
    n_tiles = WPC * T_W
    CW = CHUNK_W
    n_chunks_a = WPC // CW                      # 49
    tiles_pc = CW * T_W                         # tiles per chunk
    msg_w = n_tiles * 8 * MSGD                  # msg cols per partition

    nc = bacc.Bacc("TRN2", target_bir_lowering=False)

    msg_d = nc.declare_dram_parameter("msg", [128, msg_w], BF16, isOutput=False)
    jloc_d = nc.declare_dram_parameter("jloc", [128, n_tiles], F32, isOutput=False)
    iota_d = nc.declare_dram_parameter("iota", [128, 128], F32, isOutput=False)
    norm_d = nc.declare_dram_parameter("norm_nm", [128, WPC], F32, isOutput=False)
    xhat_d = nc.declare_dram_parameter("xhatT", [GEO, NPC], F32R, isOutput=False)
    wt0x_d = nc.declare_dram_parameter("wt0x", [GEO, 256], F32R, isOutput=False)
    a0_d = nc.declare_dram_parameter("a0", [25, 256], F32R, isOutput=False)
    wt1_d = nc.declare_dram_parameter("wt1", [256, 512], F32R, isOutput=False)
    a1_d = nc.declare_dram_parameter("a1", [25, 512], F32R, isOutput=False)
    wtnp_d = nc.declare_dram_parameter("wtnp", [512, NSC], F32R, isOutput=False)
    anp_d = nc.declare_dram_parameter("anp", [25, NSC], F32R, isOutput=False)
    wte_d = nc.declare_dram_parameter("wte", [512, 512], F32R, isOutput=False)
    wtd0_d = nc.declare_dram_parameter("wtd0", [512, 256], F32R, isOutput=False)
    ad0_d = nc.declare_dram_parameter("ad0", [25, 256], F32R, isOutput=False)
    wtd1_d = nc.declare_dram_parameter("wtd1", [256, GEO], F32R, isOutput=False)
    ad1_d = nc.declare_dram_parameter("ad1", [25, GEO], F32R, isOutput=False)
    scores_d = nc.declare_dram_parameter("scoresT", [NSC, NPC], F32, isOutput=True)
    ahnT_d = nc.dram_tensor("ahnT_buf", [MSGD + 1, NPC], F32R)
    recon_d = nc.declare_dram_parameter("reconT", [GEO, NPC], F32, isOutput=True)

    with nc.allow_low_precision(reason="fp32r matmul inputs"), \
         TileContext(nc) as tc:
        with (
            tc.tile_pool(name="const", bufs=1) as cpool,
            tc.tile_pool(name="wts", bufs=1) as wpool,
            tc.tile_pool(name="persist", bufs=1) as ppool,
        ):
            # ---- constants ----
            iota_t = cpool.tile([128, 128], F32)
            nc.sync.dma_start(out=iota_t[:], in_=iota_d[:])
            ident_t = cpool.tile([128, 128], F32)
            make_identity(nc, ident_t[:])
            jloc_t = cpool.tile([128, n_tiles], F32)
            nc.sync.dma_start(out=jloc_t[:], in_=jloc_d[:])
            norm_t = cpool.tile([128, WPC], F32)
            nc.sync.dma_start(out=norm_t[:], in_=norm_d[:])
            ones_f = cpool.tile([128, 1], F32)
            nc.vector.memset(ones_f[:], 1.0)
            ones_r = cpool.tile([128, 1], F32R)
            nc.vector.tensor_copy(out=ones_r[:], in_=ones_f[:])
            eps_t = cpool.tile([1, 1], F32)
            nc.vector.memset(eps_t[:], LN_EPS)
            onesrow_f = cpool.tile([1, 128], F32)
            nc.vector.memset(onesrow_f[:], 1.0)
            onesrow_r = cpool.tile([1, 128], F32R)
            nc.vector.tensor_copy(out=onesrow_r[:], in_=onesrow_f[:])


            # ---- weights to SBUF ----
            def load_w(dram, rows, cols):
                tiles = []
                r0 = 0
                while r0 < rows:
                    r = min(128, rows - r0)
                    t = wpool.tile([r, cols], F32R, tag=f"w{dram.name}{r0}")
                    nc.sync.dma_start(out=t[:], in_=dram[r0:r0 + r, :])
                    tiles.append((t, r))
                    r0 += r
                return tiles

            wt0x = load_w(wt0x_d, GEO, 256)
            a0 = load_w(a0_d, 25, 256)
            wt1 = load_w(wt1_d, 256, 512)
            a1 = load_w(a1_d, 25, 512)
            wtnp = load_w(wtnp_d, 512, NSC)
            anp = load_w(anp_d, 25, NSC)
            wte = load_w(wte_d, 512, 512)
            wtd0 = load_w(wtd0_d, 512, 256)
            ad0 = load_w(ad0_d, 25, 256)
            wtd1 = load_w(wtd1_d, 256, GEO)
            ad1 = load_w(ad1_d, 25, GEO)

            # ================= phase A =================
            with (
                tc.tile_pool(name="msgp", bufs=3) as mpool,
                tc.tile_pool(name="subs", bufs=3) as spool,
                tc.tile_pool(name="aggp", bufs=4) as apool,
                tc.tile_pool(name="psA", bufs=4, space="PSUM") as psA,
                tc.tile_pool(name="ahp", bufs=1) as ahpool,
            ):
                ah_t = ahpool.tile([128, WPC * MSGD], F32)
                ahn_t = ahpool.tile([128, WPC * (MSGD + 1)], F32)
                for ck in range(n_chunks_a):
                    msg_t = mpool.tile([128, tiles_pc * 8 * MSGD], BF16)
                    nc.sync.dma_start(
                        out=msg_t[:],
                        in_=msg_d[:, ck * tiles_pc * 8 * MSGD:
                                  (ck + 1) * tiles_pc * 8 * MSGD])
                    subs_t = spool.tile([128, tiles_pc * MSGD], F32R)
                    msg_v = msg_t[:].rearrange("p (t s f) -> p t f s",
                                               t=tiles_pc, s=8, f=MSGD)
                    nc.vector.reduce_sum(
                        out=subs_t[:].rearrange("p (t f) -> p t f",
                                                t=tiles_pc, f=MSGD),
                        in_=msg_v, axis=mybir.AxisListType.X)
                    win_p = psA.tile([128, CW * MSGD], F32, space="PSUM",
                                     tag="win")
                    for tt in range(tiles_pc):
                        w_in_c = tt // T_W
                        k_in_w = tt % T_W
                        agg_t = apool.tile([128, 128], F32R, tag="agg")
                        gtile = ck * tiles_pc + tt
                        nc.vector.tensor_scalar(
                            out=agg_t[:], in0=iota_t[:],
                            scalar1=jloc_t[:, gtile:gtile + 1], scalar2=None,
                            op0=mybir.AluOpType.is_equal)
                        nc.tensor.matmul(
                            out=win_p[:, w_in_c * MSGD:(w_in_c + 1) * MSGD],
                            lhsT=agg_t[:],
                            rhs=subs_t[:, tt * MSGD:(tt + 1) * MSGD],
                            start=(k_in_w == 0), stop=(k_in_w == T_W - 1))
                    nc.vector.tensor_copy(
                        out=ah_t[:, ck * CW * MSGD:(ck + 1) * CW * MSGD],
                        in_=win_p[:])

                # ---- ah * norm, ones column -> ahn [128, WPC*25] ----
                ahn_v = ahn_t[:].rearrange("p (w f) -> p w f", w=WPC,
                                           f=MSGD + 1)
                nc.vector.memset(ahn_v[:, :, 0:1], 1.0)
                normB_t = mpool.tile([128, WPC * MSGD], F32, tag="msg_t")
                nc.vector.tensor_copy(
                    out=normB_t[:].rearrange("p (w f) -> p w f", w=WPC, f=MSGD),
                    in_=norm_t[:].to_broadcast([128, WPC, MSGD]))
                nc.vector.tensor_tensor(
                    out=ahn_v[:, :, 1:MSGD + 1],
                    in0=ah_t[:].rearrange("p (w f) -> p w f", w=WPC, f=MSGD),
                    in1=normB_t[:].rearrange("p (w f) -> p w f", w=WPC, f=MSGD),
                    op=mybir.AluOpType.mult)

                # ---- transpose ahn -> ahnT [25, NPC] ----
                for g in range(0, WPC, 4):
                    gw = min(4, WPC - g)
                    tr_p = psA.tile([MSGD + 1, 512], F32, space="PSUM",
                                    tag="trp")
                    for k in range(gw):
                        nc.tensor.transpose(
                            out=tr_p[:, k * 128:(k + 1) * 128],
                            in_=ahn_v[:, g + k, :], identity=ident_t[:])
                    ncols = min(NPC - g * 128, gw * 128)
                    stg = spool.tile([MSGD + 1, 512], F32R, tag="stg")
                    nc.vector.tensor_copy(out=stg[:, 0:ncols],
                                          in_=tr_p[:, 0:ncols])
                    nc.sync.dma_start(
                        out=ahnT_d[:, g * 128:g * 128 + ncols],
                        in_=stg[:, 0:ncols])

            # ================= phase B =================
            with (
                tc.tile_pool(name="actp", bufs=2) as hpool,
                tc.tile_pool(name="psB", bufs=5, space="PSUM") as psB,
                tc.tile_pool(name="psS", bufs=1, space="PSUM") as psS,
            ):
                def layer(rhs_tiles, wts, fout, cn, ln_relu, out_dtype=F32R,
                          tag=""):
                    n_mt = (fout + 127) // 128
                    zs = []
                    for mt in range(n_mt):
                        m = min(128, fout - mt * 128)
                        z_p = psB.tile([128, NODE_CHUNK], F32, space="PSUM",
                                       tag="z")
                        for kt, ((rhs_ap, k), wt_ap) in enumerate(
                                zip(rhs_tiles, wts)):
                            nc.tensor.matmul(
                                out=z_p[:m, 0:cn],
                                lhsT=wt_ap[:, mt * 128:mt * 128 + m],
                                rhs=rhs_ap,
                                start=(kt == 0),
                                stop=(kt == len(rhs_tiles) - 1))
                        zs.append((z_p, m))
                    if not ln_relu:
                        outs = []
                        for mt, (z_p, m) in enumerate(zs):
                            o = hpool.tile([128, NODE_CHUNK], out_dtype,
                                           tag=f"o{tag}{mt}")
                            nc.scalar.copy(out=o[:m, 0:cn], in_=z_p[:m, 0:cn])
                            outs.append((o, m))
                        return outs
                    var_p = psS.tile([1, NODE_CHUNK], F32, space="PSUM",
                                     tag="v")
                    zsqs = []
                    for mt, (z_p, m) in enumerate(zs):
                        zsq = hpool.tile([128, NODE_CHUNK], F32R, tag="q")
                        nc.scalar.square(out=zsq[:m, 0:cn], in_=z_p[:m, 0:cn])
                        zsqs.append((zsq, m))
                    for mt, (zsq, m) in enumerate(zsqs):
                        nc.tensor.matmul(
                            out=var_p[:, 0:cn], lhsT=ones_r[:m, :],
                            rhs=zsq[:m, 0:cn],
                            start=(mt == 0), stop=(mt == len(zsqs) - 1))
                    s_t = hpool.tile([1, NODE_CHUNK], F32R, tag="s")
                    nc.scalar.activation(
                        out=s_t[:, 0:cn], in_=var_p[:, 0:cn],
                        func=mybir.ActivationFunctionType.Sqrt,
                        bias=eps_t[:1, :1], scale=1.0 / fout)
                    sB_p = psS.tile([128, NODE_CHUNK], F32, space="PSUM",
                                    tag="sb")
                    nc.tensor.matmul(out=sB_p[:, 0:cn], lhsT=onesrow_r[:, :],
                                     rhs=s_t[:, 0:cn], start=True, stop=True)
                    rstdB = hpool.tile([128, NODE_CHUNK], F32, tag="r")
                    nc.vector.reciprocal(out=rstdB[:, 0:cn], in_=sB_p[:, 0:cn])
                    outs = []
                    for mt, (z_p, m) in enumerate(zs):
                        t_t = hpool.tile([128, NODE_CHUNK], F32, tag="t")
                        nc.vector.tensor_tensor(
                            out=t_t[:m, 0:cn], in0=z_p[:m, 0:cn],
                            in1=rstdB[:m, 0:cn],
                            op=mybir.AluOpType.mult)
                        o = hpool.tile([128, NODE_CHUNK], out_dtype,
                                       tag=f"o{tag}{mt}")
                        if ln_relu == "relu":
                            nc.scalar.activation(
                                out=o[:m, 0:cn], in_=t_t[:m, 0:cn],
                                func=mybir.ActivationFunctionType.Relu)
                        else:
                            nc.scalar.copy(out=o[:m, 0:cn], in_=t_t[:m, 0:cn])
                        outs.append((o, m))
                    return outs

                n_chunks_b = (NPC + NODE_CHUNK - 1) // NODE_CHUNK
                for nck in range(n_chunks_b):
                    c0 = nck * NODE_CHUNK
                    cn = min(NODE_CHUNK, NPC - c0)
                    sl = slice(c0, c0 + cn)
                    ahn_c = hpool.tile([MSGD + 1, NODE_CHUNK], F32R, tag="ahc")
                    nc.sync.dma_start(out=ahn_c[:, 0:cn], in_=ahnT_d[:, sl])
                    xhat_c = hpool.tile([GEO, NODE_CHUNK], F32R, tag="xhc")
                    nc.sync.dma_start(out=xhat_c[:, 0:cn], in_=xhat_d[:, sl])
                    ah25 = ahn_c[0:MSGD + 1, 0:cn]

                    h1 = layer([(xhat_c[:, 0:cn], GEO), (ah25, MSGD + 1)],
                               [wt0x[0][0], a0[0][0]],
                               256, cn, "relu", tag="e0")
                    h2 = layer([(h1[0][0][:, 0:cn], 128),
                                (h1[1][0][:, 0:cn], 128),
                                (ah25, MSGD + 1)],
                               [wt1[0][0], wt1[1][0], a1[0][0]],
                               512, cn, "relu", tag="e1")
                    h2r = [(t[:, 0:cn], 128) for t, m in h2]
                    sc = layer(h2r + [(ah25, MSGD + 1)],
                               [wtnp[0][0], wtnp[1][0], wtnp[2][0],
                                wtnp[3][0], anp[0][0]],
                               NSC, cn, "ln", out_dtype=F32, tag="np")
                    nc.sync.dma_start(out=scores_d[:, sl],
                                      in_=sc[0][0][0:NSC, 0:cn])
                    rep = layer(h2r,
                                [wte[0][0], wte[1][0], wte[2][0], wte[3][0]],
                                512, cn, False, tag="ed")
                    repr_ = [(t[:, 0:cn], 128) for t, m in rep]
                    d1 = layer(repr_ + [(ah25, MSGD + 1)],
                               [wtd0[0][0], wtd0[1][0], wtd0[2][0],
                                wtd0[3][0], ad0[0][0]],
                               256, cn, "relu", tag="d0")
                    rec = layer([(d1[0][0][:, 0:cn], 128),
                                 (d1[1][0][:, 0:cn], 128),
                                 (ah25, MSGD + 1)],
                                [wtd1[0][0], wtd1[1][0], ad1[0][0]],
                                GEO, cn, "relu", out_dtype=F32, tag="d1")
                    nc.sync.dma_start(out=recon_d[:, sl],
                                      in_=rec[0][0][0:GEO, 0:cn])

    nc.compile()
    return nc


# --------------------------------------------------------------------------
# entry point
# --------------------------------------------------------------------------

def kernel(**inputs):
    import os
    per_core, w, meta = _prepare(inputs)
    T_W = meta["T_W"]
    if T_W not in _PROG_CACHE:
        _PROG_CACHE[T_W] = _build_program(T_W)
    nc = _PROG_CACHE[T_W]

    in_maps = []
    for c in range(NCORES):
        m = dict(per_core[c])
        m.update(w)
        in_maps.append(m)

    import time as _time
    trace = bool(int(os.environ.get("BASS_KERNEL_TRACE", "0")))
    t0 = _time.time()
    try:
        res = run_bass_kernel_spmd(nc, in_maps, list(range(NCORES)), trace=trace)
    except ModuleNotFoundError:
        res = run_bass_kernel_spmd(nc, in_maps, list(range(NCORES)), trace=False)
    exec_wall = _time.time() - t0
    if getattr(res, "exec_time_ns", None) is not None:
        print(f"HW exec time: {res.exec_time_ns} ns")
    else:
        print(f"HW exec time: {int(exec_wall * 1e9)} ns (wall-clock of run, no profile)")

    scores = np.concatenate([res.results[c]["scoresT"].T for c in range(NCORES)],
                            axis=0)
    recon = np.concatenate([res.results[c]["reconT"].T for c in range(NCORES)],
                           axis=0)
    g = np.asarray(inputs["np_g"], np.float32)
    beta = np.asarray(inputs["np_beta"], np.float32)
    n_scores = scores * g[None, :] + beta[None, :]
    mask_nodes = np.asarray(inputs["mask_nodes"], np.int64)
    x = np.asarray(inputs["x"], np.float32)
    x_pred = recon[mask_nodes].astype(np.float32)
    x_true = x[mask_nodes].astype(np.float32)
    return (x_pred, x_true, n_scores.astype(np.float32))
